# revision 11
# baseline (speedup 1.0000x reference)
"""Trainium2 Bass kernel for nn_DIDAModuleD4 (dynamic depthwise conv module).

Data-parallel over batch: 32 samples -> 8 cores x 4 samples.
Per core, samples are processed in 2 blocks of 2 samples; each block maps the
2x64=128 (sample, channel) pairs onto the 128 SBUF partitions.

Math (per sample, with host-side weight folding):
  f   = relu(conv_w @ x + conv_b)                       [64, 4096]
  g   = relu(mean_px(conv_w @ x + conv_b))              [64]
  k_t = a_t * g + b_t            (43 taps, a/b host-folded scalars)
  o_i = sum_t k_t * shift_t(f)   (depthwise; 5x5, 3x3 d2, 3x3 d4)
  out = sum_i W_i @ o_i + bias_out                      [384, 4096]

All 43 depthwise taps run on the PE as fp8e4 DoubleRow matmuls, two taps per
matmul: f is stored as an fp8 padded tile (72x72 flat domain) plus 6 shifted
copies (SBUF->SBUF DMA, shifts 1/2/4/72/144/288); a tap pair (t, t+delta)
reads k-tiles (slot0@off, slot_delta@off) so the rhs AP is [p, 2, N] with a
large monotonic dim-1 stride (small strides fault the PE).  lhsT k-tiles are
fp8 diag(k_t) matrices built per block from ktile; odd taps pair with an
all-zero diag slot.  DoubleRow costs 0.5 PE cycles/output-column for 2 taps
vs 1.0 for one bf16 tap (4x).  Tap matmuls produce 7-row x 72-col psum
chunks; the 8 pad columns per row are skipped at evacuation (pad wraparound
reads land in neighbor-row pad zeros since |dx*dil| <= PAD).

k values (~2.5e-3) sit in fp8's subnormal range, so ktile is prescaled by a
host-computed power of two per branch and the branch evacuation divides it
back out.  conv (f32r block-diag, 2-sample) and output 1x1s (bf16) are as in
the bf16 baseline; evacuations are spread across ACT/DVE/Pool.
"""

import sys

if "/opt/trn_rl_repo" not in sys.path:
    sys.path.insert(0, "/opt/trn_rl_repo")

import numpy as np
from contextlib import ExitStack

from concourse import bass, mybir, tile, bacc
from concourse.bass_utils import run_bass_kernel_spmd

F32 = mybir.dt.float32
F32R = mybir.dt.float32r
BF16 = mybir.dt.bfloat16
FP8 = mybir.dt.float8e4
AF = mybir.ActivationFunctionType
ALU = mybir.AluOpType
DRMODE = mybir.MatmulPerfMode.DoubleRow

N_CORES = 8
SAMPLES_PER_CORE = 4
CM = 64
CIN = 256
COUT = 384
H = W = 64
PIX = H * W          # 4096
PAD = 4
WP = W + 2 * PAD     # 72
FSZ = WP * WP        # 5184
GUARD = 4            # front/back guard elems per FF slot (OOB garbage ok)
SLP = FSZ + 2 * GUARD  # 5192 slot pitch
SHIFTS = (0, 1, 2, 4, WP, 2 * WP, 4 * WP)   # FF slot shifts
SHIFT_SLOT = {s: i for i, s in enumerate(SHIFTS)}
NFF = len(SHIFTS)    # 7
HALF = 2048          # pixels per half (32 rows)
CHUNK = 512          # conv/out matmul N
SLAB = 1024
NSLAB = PIX // SLAB
# tap-psum chunks per half: 7-row pieces of the 32 rows
TAPCH = ((0, 7), (7, 7), (14, 7), (21, 7), (28, 4))

# taps: (branch, dy, dx); dil = (1, 2, 4)[branch]
DILS = (1, 2, 4)


def _tap_pairs():
    """Pair taps so each pair's flat-offset delta is one of SHIFTS[1:].
    Returns list of (t1, t2_or_None, shift) with t=(br, dy, dx);
    t2's flat offset == t1's + shift (shift==0 for singles)."""
    pairs = []
    # b0 (5x5, dil 1)
    for dy in range(-2, 3):
        pairs.append(((0, dy, -2), (0, dy, -1), 1))
        pairs.append(((0, dy, 0), (0, dy, 1), 1))
    pairs.append(((0, -2, 2), (0, -1, 2), WP))
    pairs.append(((0, 0, 2), (0, 1, 2), WP))
    pairs.append(((0, 2, 2), None, 0))
    # b1 (3x3, dil 2)
    for dy in range(-1, 2):
        pairs.append(((1, dy, -1), (1, dy, 0), 2))
    pairs.append(((1, -1, 1), (1, 0, 1), 2 * WP))
    pairs.append(((1, 1, 1), None, 0))
    # b2 (3x3, dil 4)
    for dy in range(-1, 2):
        pairs.append(((2, dy, -1), (2, dy, 0), 4))
    pairs.append(((2, -1, 1), (2, 0, 1), 4 * WP))
    pairs.append(((2, 1, 1), None, 0))
    return pairs


PAIRS = _tap_pairs()
NSLOT = 2 * len(PAIRS)          # diag bank slots (46)
BR_PAIRS = {br: [(j, p) for j, p in enumerate(PAIRS) if p[0][0] == br]
            for br in range(3)}
assert [len(BR_PAIRS[b]) for b in range(3)] == [13, 5, 5]


def _tap_off(t, row0):
    """Flat offset (within a slot, before the +GUARD base) of tap t's rhs
    for an output chunk starting at block row `row0`, extended col 0."""
    br, dy, dx = t
    dil = DILS[br]
    return (PAD + row0 + dy * dil) * WP + dx * dil


_PROGRAM_CACHE = {}


def _build_program():
    nc = bacc.Bacc("TRN2", target_bir_lowering=False, debug=False,
                   num_devices=N_CORES)

    x4 = nc.dram_tensor("x4", [SAMPLES_PER_CORE, CIN, PIX], BF16,
                        kind="ExternalInput").ap()
    wconv = nc.dram_tensor("wconv", [128, 512], BF16,
                           kind="ExternalInput").ap()
    wout12_d = nc.dram_tensor("wout12", [128, 3 * 128], BF16,
                              kind="ExternalInput").ap()
    wout1_d = nc.dram_tensor("wout1", [128, COUT], BF16,
                             kind="ExternalInput").ap()
    aT_d = nc.dram_tensor("aT", [128, NSLOT], F32, kind="ExternalInput").ap()
    bT_d = nc.dram_tensor("bT", [128, NSLOT], F32, kind="ExternalInput").ap()
    ident_d = nc.dram_tensor("ident", [128, 128], FP8,
                             kind="ExternalInput").ap()
    convb_d = nc.dram_tensor("convb", [128, 1], F32, kind="ExternalInput").ap()
    biasout_d = nc.dram_tensor("biasout", [128, 3], F32,
                               kind="ExternalInput").ap()
    # per-branch inverse tap scales (folded into branch evac)
    sinv_d = nc.dram_tensor("sinv", [128, 3], F32, kind="ExternalInput").ap()
    y4 = nc.dram_tensor("y4", [SAMPLES_PER_CORE, COUT, PIX], F32,
                        kind="ExternalOutput").ap()

    with tile.TileContext(nc) as tc:
        with ExitStack() as ctx:
            consts = ctx.enter_context(tc.tile_pool(name="consts", bufs=1))
            xpool = ctx.enter_context(tc.tile_pool(name="xp", bufs=3))
            ffpool = ctx.enter_context(tc.tile_pool(name="ffp", bufs=1))
            dgpool = ctx.enter_context(tc.tile_pool(name="dgp", bufs=1))
            opool = ctx.enter_context(tc.tile_pool(name="op", bufs=2))
            outpool = ctx.enter_context(tc.tile_pool(name="outp", bufs=2))
            smalls = ctx.enter_context(tc.tile_pool(name="sm", bufs=2))
            ps_a = ctx.enter_context(
                tc.tile_pool(name="psa", bufs=1, space="PSUM"))
            ps_out = ctx.enter_context(
                tc.tile_pool(name="pso", bufs=1, space="PSUM"))

            # ---- constants ----
            wconv_t = consts.tile([128, 512], BF16, tag="wconv")
            nc.gpsimd.dma_start(wconv_t[:], wconv[:])
            wout12_t = consts.tile([128, 3 * 128], BF16, tag="wout12")
            nc.gpsimd.dma_start(wout12_t[:], wout12_d[:])
            wout1_t = consts.tile([128, COUT], BF16, tag="wout1")
            nc.gpsimd.dma_start(wout1_t[:], wout1_d[:])
            aT = consts.tile([128, NSLOT], F32, tag="aT")
            nc.gpsimd.dma_start(aT[:], aT_d[:])
            bT = consts.tile([128, NSLOT], F32, tag="bT")
            nc.gpsimd.dma_start(bT[:], bT_d[:])
            ident = consts.tile([128, 128], FP8, tag="ident")
            nc.gpsimd.dma_start(ident[:], ident_d[:])
            convb = consts.tile([128, 1], F32, tag="convb")
            nc.gpsimd.dma_start(convb[:], convb_d[:])
            biasout = consts.tile([128, 3], F32, tag="biasout")
            nc.gpsimd.dma_start(biasout[:], biasout_d[:])
            sinv = consts.tile([128, 3], F32, tag="sinv")
            nc.gpsimd.dma_start(sinv[:], sinv_d[:])

            # persistent FF tiles (one per block parity): 7 slots of padded
            # fp8 f (slot 0) and its shifted copies; pad borders zeroed once.
            ffs, banks = [], []
            zeros = consts.tile([128, PAD * WP], F32, tag="zeros")
            nc.gpsimd.memset(zeros[:], 0.0)
            for par in range(2):
                ff = ffpool.tile([128, NFF * SLP], FP8, tag=f"ff{par}")
                s0 = ff[:, GUARD:GUARD + FSZ]
                v = s0.rearrange("p (r c) -> p r c", c=WP)
                nc.vector.tensor_copy(s0[:, 0:PAD * WP], zeros[:])
                nc.vector.tensor_copy(s0[:, (PAD + H) * WP:FSZ], zeros[:])
                zv = zeros[:].rearrange("p (r c) -> p r c", c=PAD)
                nc.vector.tensor_copy(v[:, PAD:PAD + H, 0:PAD], zv[:, 0:H, :])
                nc.vector.tensor_copy(v[:, PAD:PAD + H, PAD + W:WP],
                                      zv[:, 0:H, :])
                # zero guards + shifted-slot tails once: stale SBUF there can
                # be Inf/NaN bit patterns, and 0 * Inf = NaN even through a
                # zero diag k-tile
                nc.gpsimd.memset(ff[:, 0:GUARD], 0.0)
                nc.gpsimd.memset(ff[:, GUARD + FSZ:SLP], 0.0)
                for si in range(1, NFF):
                    nc.gpsimd.memset(
                        ff[:, si * SLP + SLP - SHIFTS[si]:(si + 1) * SLP],
                        0.0)
                ffs.append(ff)
                bank = dgpool.tile([128, NSLOT, 128], FP8, tag=f"bank{par}")
                nc.gpsimd.memset(bank[:], 0.0)
                banks.append(bank)

            def ff_slot0_rows(ff, r0, nrows):
                """[p, nrows, 64] valid-interior view of slot0 (conv dst)."""
                v = ff[:, GUARD:GUARD + FSZ].rearrange("p (r c) -> p r c",
                                                       c=WP)
                return v[:, PAD + r0:PAD + r0 + nrows, PAD:PAD + W]

            def pair_rhs(ff, t1, shift, row0, ncols):
                """[p, 2, ncols] rhs AP: k-tile1 = slot0 @ off(t1),
                k-tile2 = slot(shift) @ same off (== f @ off+shift)."""
                off = GUARD + _tap_off(t1, row0)
                v = ff[:, off:off + ncols]
                u = v.unsqueeze(1).broadcast_to([128, 2, ncols])
                # singles (shift 0) pair with the zero diag; point k-tile2
                # at slot 1 — small or zero dim-1 strides fault the PE
                slot = SHIFT_SLOT[shift] or 1
                u.ap[1] = [slot * SLP, 2]
                return u

            # round-robin engine pickers for evac/diag work
            def rr(seq):
                i = [0]

                def pick():
                    e = seq[i[0] % len(seq)]
                    i[0] += 1
                    return e
                return pick

            # Pool/GPSIMD cannot read PSUM: psum evacs go to ACT/DVE only;
            # Pool absorbs SBUF-side work (diags, relu) instead.
            diag_eng = rr(["g", "v", "g", "g", "v"])
            evac_eng = rr(["a", "v"])
            out_eng = rr(["a", "v"])

            def emit_conv(blk):
                n0 = 2 * blk
                ff = ffs[blk % 2]
                bank = banks[blk % 2]

                gsums = smalls.tile([128, 8], F32, tag=f"gsums{blk}")
                for q in range(NSLAB):
                    xts = {}
                    for kc in range(4):
                        xt = xpool.tile([128, SLAB], BF16, tag=f"x{kc}")
                        nc.sync.dma_start(
                            xt[:],
                            x4[n0:n0 + 2, kc * 64:(kc + 1) * 64,
                               q * SLAB:(q + 1) * SLAB])
                        xts[kc] = xt
                    for c in range(SLAB // CHUNK):
                        j = q * (SLAB // CHUNK) + c
                        ps = ps_a.tile([128, CHUNK], F32, tag=f"tap{j % 2}")
                        for kc in range(4):
                            nc.tensor.matmul(
                                ps[:],
                                wconv_t[:, kc * 128:(kc + 1) * 128],
                                xts[kc][:, c * CHUNK:(c + 1) * CHUNK],
                                start=(kc == 0), stop=(kc == 3))
                        dst = ff_slot0_rows(ff, 8 * j, 8)
                        nc.scalar.activation(
                            dst, ps[:], AF.Identity,
                            bias=convb[:, 0:1],
                            accum_out=gsums[:, j:j + 1])

                # relu in place over the full slot0 (pads stay 0), split
                # between DVE and Pool
                s0 = ff[:, GUARD:GUARD + FSZ]
                hf = FSZ // 2
                nc.vector.tensor_scalar_max(s0[:, 0:hf], s0[:, 0:hf], 0.0)
                nc.gpsimd.tensor_scalar_max(s0[:, hf:FSZ], s0[:, hf:FSZ], 0.0)

                # shifted copies via SBUF->SBUF DMA; issued from the Pool
                # SWDGE queue so the SP sequencer keeps prefetching x slabs
                for si in range(1, NFF):
                    sh = SHIFTS[si]
                    nc.gpsimd.dma_start(
                        ff[:, si * SLP:si * SLP + SLP - sh],
                        ff[:, sh:SLP])

                # g -> ktile -> diag bank
                gpre = smalls.tile([128, 1], F32, tag=f"gpre{blk}")
                nc.vector.tensor_reduce(gpre[:], gsums[:], op=ALU.add,
                                        axis=mybir.AxisListType.X)
                gt = smalls.tile([128, 1], F32, tag=f"g{blk}")
                nc.scalar.activation(gt[:], gpre[:], AF.Relu,
                                     scale=1.0 / PIX)
                ktile = smalls.tile([128, NSLOT], F32, tag=f"ktile{blk}")
                nc.vector.scalar_tensor_tensor(ktile[:], aT[:], gt[:, 0:1],
                                               bT[:], op0=ALU.mult,
                                               op1=ALU.add)
                nc.vector.tensor_scalar_min(ktile[:], ktile[:], 240.0)
                nc.vector.tensor_scalar_max(ktile[:], ktile[:], -240.0)

                for j, (t1, t2, _sh) in enumerate(PAIRS):
                    for half_, t in ((0, t1), (1, t2)):
                        if t is None:
                            continue
                        sl = 2 * j + half_
                        e = diag_eng()
                        if e == "v":
                            nc.vector.tensor_scalar_mul(
                                bank[:, sl, :], ident[:],
                                ktile[:, sl:sl + 1])
                        elif e == "g":
                            nc.gpsimd.tensor_scalar_mul(
                                bank[:, sl, :], ident[:],
                                ktile[:, sl:sl + 1])
                        else:
                            nc.scalar.activation(
                                bank[:, sl, :], ident[:], AF.Copy,
                                scale=ktile[:, sl:sl + 1])
                return (ff, bank, n0)

            def emit_stage(st, h):
                """One (blk, h): tap chunks and out chunks interleaved so PE
                always has matmul work while ACT/DVE drain psums.
                Interleave: t0 t1 o0 t2 o1 t3 o2 t4 o3 (out chunk oc needs
                branch cols through 512(oc+1), covered by tap chunks
                through ceil(512(oc+1)/448)-1)."""
                ff, bank, n0 = st
                o1t = opool.tile([128, HALF], BF16, tag="o1")
                Xt = opool.tile([128, HALF], BF16, tag="X")
                Yt = opool.tile([128, HALF], BF16, tag="Y")
                pi = [0]
                osbs = {}

                def tap_chunk(ci):
                    lr0, nr = TAPCH[ci]
                    row0 = 32 * h + lr0
                    ncols = nr * WP
                    for br in range(3):
                        ps = ps_a.tile([128, 512], F32,
                                       tag=f"tap{pi[0] % 2}")
                        pi[0] += 1
                        plist = BR_PAIRS[br]
                        for i, (j, (t1, t2, sh)) in enumerate(plist):
                            rhs = pair_rhs(ff, t1, sh, row0, ncols)
                            nc.tensor.matmul(
                                ps[:, 0:ncols], bank[:, 2 * j:2 * j + 2, :],
                                rhs, start=(i == 0),
                                stop=(i == len(plist) - 1),
                                perf_mode=DRMODE)
                        src = ps[:, 0:ncols].rearrange(
                            "p (r c) -> p r c", c=WP)[:, :, PAD:PAD + W]
                        c0 = lr0 * W
                        csl = slice(c0, c0 + nr * W)
                        if br == 0:
                            dsts = [(slice(0, 128), o1t[:, csl])]
                        elif br == 1:
                            dsts = [(slice(0, 64), Xt[0:64, csl]),
                                    (slice(64, 128), Yt[0:64, csl])]
                        else:
                            dsts = [(slice(0, 64), Xt[64:128, csl]),
                                    (slice(64, 128), Yt[64:128, csl])]
                        for psl, dst in dsts:
                            e = evac_eng()
                            sc = sinv[psl, br:br + 1]
                            if e == "a":
                                nc.scalar.activation(dst, src[psl], AF.Copy,
                                                     scale=sc)
                            else:
                                nc.vector.tensor_scalar_mul(dst, src[psl],
                                                            sc)

                def out_chunk(oc):
                    if oc % 2 == 0:
                        for mt in range(3):
                            for s in range(2):
                                osb_tile = outpool.tile(
                                    [128, 2 * CHUNK], F32,
                                    tag=f"osb{mt}_{s}")
                                osbs[(mt, s)] = osb_tile
                    csl = slice(oc * CHUNK, (oc + 1) * CHUNK)
                    pss = {}
                    for mt in range(3):
                        for s, bt in ((0, Xt), (1, Yt)):
                            ps = ps_out.tile([128, CHUNK], F32,
                                             tag=f"out{s}_{mt}")
                            pss[(s, mt)] = ps
                            nc.tensor.matmul(
                                ps[:],
                                wout12_t[:, mt * 128:(mt + 1) * 128],
                                bt[:, csl], start=True, stop=False)
                    for mt in range(3):
                        for s in range(2):
                            sl = slice(64 * s, 64 * s + 64)
                            nc.tensor.matmul(
                                pss[(s, mt)][:],
                                wout1_t[sl, mt * 128:(mt + 1) * 128],
                                o1t[sl, csl], start=False, stop=True)
                    for mt in range(3):
                        for s in range(2):
                            dst = osbs[(mt, s)][:, (oc % 2) * CHUNK:
                                                (oc % 2 + 1) * CHUNK]
                            if out_eng() == "a":
                                nc.scalar.activation(
                                    dst, pss[(s, mt)][:], AF.Identity,
                                    bias=biasout[:, mt:mt + 1])
                            else:
                                nc.vector.scalar_tensor_tensor(
                                    dst, pss[(s, mt)][:], 1.0,
                                    biasout[:, mt:mt + 1]
                                    .broadcast_to([128, CHUNK]),
                                    op0=ALU.mult, op1=ALU.add)
                    if oc % 2 == 1:
                        g2 = oc // 2
                        px0 = h * HALF + g2 * 2 * CHUNK
                        for mt in range(3):
                            for s in range(2):
                                dst = y4[n0 + s, mt * 128:(mt + 1) * 128,
                                         px0:px0 + 2 * CHUNK]
                                if g2 == 0:
                                    nc.gpsimd.dma_start(dst,
                                                        osbs[(mt, s)][:])
                                else:
                                    nc.scalar.dma_start(dst,
                                                        osbs[(mt, s)][:])

                for kind, i in (("t", 0), ("t", 1), ("o", 0), ("t", 2),
                                ("o", 1), ("t", 3), ("o", 2), ("t", 4),
                                ("o", 3)):
                    if kind == "t":
                        tap_chunk(i)
                    else:
                        out_chunk(i)

            st0 = emit_conv(0)
            emit_stage(st0, 0)
            st1 = emit_conv(1)
            emit_stage(st0, 1)
            emit_stage(st1, 0)
            emit_stage(st1, 1)
    nc.compile()
    return nc


def _get_program():
    if "nc" not in _PROGRAM_CACHE:
        _PROGRAM_CACHE["nc"] = _build_program()
    return _PROGRAM_CACHE["nc"]


def kernel(x, conv_w, conv_b, ck_w, ck_b, ck2_w, ck2_b, ckd4_w, ckd4_b,
           kern_w, kern_b, kern2_w, kern2_b, kernd4_w, kernd4_b,
           fuse_w, fuse_b, fc_w, fc_b):
    import ml_dtypes
    x = np.asarray(x, dtype=np.float32)
    conv_w = np.asarray(conv_w, dtype=np.float32)
    conv_b = np.asarray(conv_b, dtype=np.float32)
    fuse_w = np.asarray(fuse_w, dtype=np.float32)
    fuse_b = np.asarray(fuse_b, dtype=np.float32)
    fc_w = np.asarray(fc_w, dtype=np.float32)
    fc_b = np.asarray(fc_b, dtype=np.float32)

    NB = x.shape[0]
    assert NB == N_CORES * SAMPLES_PER_CORE

    # tap affine coefficients per branch: k_t = a_t * g + b_t
    def fold(sw, sb, kw, kb):
        a = (float(sw) * np.asarray(kw)).astype(np.float32)
        b = (float(sw) * np.asarray(kb) + float(sb)).astype(np.float32)
        return a, b

    a1, b1 = fold(ck_w, ck_b, kern_w, kern_b)        # [25], 5x5 row-major
    a2, b2 = fold(ck2_w, ck2_b, kern2_w, kern2_b)    # [9]
    a3, b3 = fold(ckd4_w, ckd4_b, kernd4_w, kernd4_b)

    def coef(t):
        br, dy, dx = t
        if br == 0:
            return a1[(dy + 2) * 5 + (dx + 2)], b1[(dy + 2) * 5 + (dx + 2)]
        a, b = (a2, b2) if br == 1 else (a3, b3)
        return a[(dy + 1) * 3 + (dx + 1)], b[(dy + 1) * 3 + (dx + 1)]

    # per-branch power-of-2 prescale: bound |k| with g <= GMAX, keep
    # S*|k| <= 200 so fp8e4m3 never saturates
    GMAX = 1.0
    scales = []
    for br in range(3):
        taps = [coef(t1) for (t1, t2, _s) in PAIRS if t1[0] == br]
        taps += [coef(t2) for (t1, t2, _s) in PAIRS
                 if t2 is not None and t2[0] == br]
        bound = max(abs(a) * GMAX + abs(b) for a, b in taps)
        scales.append(2.0 ** np.floor(np.log2(200.0 / max(bound, 1e-30))))
    sinv = np.zeros((128, 3), np.float32)
    for br in range(3):
        sinv[:, br] = 1.0 / scales[br]

    aT = np.zeros((128, NSLOT), np.float32)
    bT = np.zeros((128, NSLOT), np.float32)
    for j, (t1, t2, _sh) in enumerate(PAIRS):
        for half_, t in ((0, t1), (1, t2)):
            if t is None:
                continue
            a, b = coef(t)
            s = scales[t[0]]
            aT[:, 2 * j + half_] = a * s
            bT[:, 2 * j + half_] = b * s

    # folded output weights W_i = fc_w[:, 128i:128(i+1)] @ fuse_w  [384, 64]
    Wi = [fc_w[:, 128 * i:128 * (i + 1)] @ fuse_w for i in range(3)]
    wout12 = np.zeros((128, 3 * 128), dtype=np.float32)
    wout12[0:64, :] = Wi[1].T.reshape(64, COUT)
    wout12[64:128, :] = Wi[2].T.reshape(64, COUT)
    wout12 = wout12.astype(ml_dtypes.bfloat16)
    wout1 = np.zeros((128, COUT), dtype=np.float32)
    wout1[0:64, :] = Wi[0].T
    wout1[64:128, :] = Wi[0].T
    wout1 = wout1.astype(ml_dtypes.bfloat16)
    bias_out = (fc_w @ np.tile(fuse_b, 3) + fc_b).astype(np.float32)
    biasout = bias_out.reshape(3, 128).T.copy()

    wconv = np.zeros((128, 512), dtype=np.float32)
    for kc in range(4):
        wt = conv_w[:, 64 * kc:64 * (kc + 1)].T
        wconv[0:64, 128 * kc:128 * kc + 64] = wt
        wconv[64:128, 128 * kc + 64:128 * (kc + 1)] = wt
    wconv = wconv.astype(ml_dtypes.bfloat16)

    convb = np.concatenate([conv_b, conv_b]).reshape(128, 1).astype(np.float32)
    ident = np.eye(128, dtype=np.float32).astype(ml_dtypes.float8_e4m3)

    nc = _get_program()
    in_maps = []
    xbf = x.reshape(NB, CIN, PIX).astype(ml_dtypes.bfloat16)
    for core in range(N_CORES):
        xs = xbf[core * SAMPLES_PER_CORE:(core + 1) * SAMPLES_PER_CORE]
        in_maps.append({
            "x4": np.ascontiguousarray(xs),
            "wconv": wconv, "wout12": wout12, "wout1": wout1,
            "aT": aT, "bT": bT, "ident": ident, "convb": convb,
            "biasout": biasout, "sinv": sinv,
        })
    res = run_bass_kernel_spmd(nc, in_maps, list(range(N_CORES)))
    out = np.empty((NB, COUT, H, W), dtype=np.float32)
    for core in range(N_CORES):
        out[core * SAMPLES_PER_CORE:(core + 1) * SAMPLES_PER_CORE] = (
            res.results[core]["y4"].reshape(SAMPLES_PER_CORE, COUT, H, W))
    return out


# revision 13
# speedup vs baseline: 1.0263x; 1.0263x over previous
"""Trainium2 Bass kernel for nn_DIDAModuleD4 (dynamic depthwise conv module).

Data-parallel over batch: 32 samples -> 8 cores x 4 samples.
Per core, samples are processed in 2 blocks of 2 samples; each block maps the
2x64=128 (sample, channel) pairs onto the 128 SBUF partitions.

Math (per sample, with host-side weight folding):
  f   = relu(conv_w @ x + conv_b)                       [64, 4096]
  g   = relu(mean_px(conv_w @ x + conv_b))              [64]
  k_t = a_t * g + b_t            (43 taps, a/b host-folded scalars)
  o_i = sum_t k_t * shift_t(f)   (depthwise; 5x5, 3x3 d2, 3x3 d4)
  out = sum_i W_i @ o_i + bias_out                      [384, 4096]

All 43 depthwise taps run on the PE as fp8e4 DoubleRow matmuls, two taps per
matmul: f is stored as an fp8 padded tile (72x72 flat domain) plus 6 shifted
copies (SBUF->SBUF DMA, shifts 1/2/4/72/144/288); a tap pair (t, t+delta)
reads k-tiles (slot0@off, slot_delta@off) so the rhs AP is [p, 2, N] with a
large monotonic dim-1 stride (small strides fault the PE).  lhsT k-tiles are
fp8 diag(k_t) matrices built per block from ktile; odd taps pair with an
all-zero diag slot.  DoubleRow costs 0.5 PE cycles/output-column for 2 taps
vs 1.0 for one bf16 tap (4x).  Tap matmuls produce 7-row x 72-col psum
chunks; the 8 pad columns per row are skipped at evacuation (pad wraparound
reads land in neighbor-row pad zeros since |dx*dil| <= PAD).

k values (~2.5e-3) sit in fp8's subnormal range, so ktile is prescaled by a
host-computed power of two per branch and the branch evacuation divides it
back out.  conv (f32r block-diag, 2-sample) and output 1x1s (bf16) are as in
the bf16 baseline; evacuations are spread across ACT/DVE/Pool.
"""

import sys

if "/opt/trn_rl_repo" not in sys.path:
    sys.path.insert(0, "/opt/trn_rl_repo")

import numpy as np
from contextlib import ExitStack

from concourse import bass, mybir, tile, bacc
from concourse.bass_utils import run_bass_kernel_spmd

F32 = mybir.dt.float32
F32R = mybir.dt.float32r
BF16 = mybir.dt.bfloat16
FP8 = mybir.dt.float8e4
AF = mybir.ActivationFunctionType
ALU = mybir.AluOpType
DRMODE = mybir.MatmulPerfMode.DoubleRow

N_CORES = 8
SAMPLES_PER_CORE = 4
CM = 64
CIN = 256
COUT = 384
H = W = 64
PIX = H * W          # 4096
PAD = 4
WP = W + 2 * PAD     # 72
FSZ = WP * WP        # 5184
GUARD = 4            # front/back guard elems per FF slot (OOB garbage ok)
SLP = FSZ + 2 * GUARD  # 5192 slot pitch
SHIFTS = (0, 1, 2, 4, WP, 2 * WP, 4 * WP)   # FF slot shifts
SHIFT_SLOT = {s: i for i, s in enumerate(SHIFTS)}
NFF = len(SHIFTS)    # 7
HALF = 2048          # pixels per half (32 rows)
CHUNK = 512          # conv/out matmul N
SLAB = 1024
NSLAB = PIX // SLAB
# tap-psum chunks per half: 7-row pieces of the 32 rows
TAPCH = ((0, 7), (7, 7), (14, 7), (21, 7), (28, 4))

# taps: (branch, dy, dx); dil = (1, 2, 4)[branch]
DILS = (1, 2, 4)


def _tap_pairs():
    """Pair taps so each pair's flat-offset delta is one of SHIFTS[1:].
    Returns list of (t1, t2_or_None, shift) with t=(br, dy, dx);
    t2's flat offset == t1's + shift (shift==0 for singles)."""
    pairs = []
    # b0 (5x5, dil 1)
    for dy in range(-2, 3):
        pairs.append(((0, dy, -2), (0, dy, -1), 1))
        pairs.append(((0, dy, 0), (0, dy, 1), 1))
    pairs.append(((0, -2, 2), (0, -1, 2), WP))
    pairs.append(((0, 0, 2), (0, 1, 2), WP))
    pairs.append(((0, 2, 2), None, 0))
    # b1 (3x3, dil 2)
    for dy in range(-1, 2):
        pairs.append(((1, dy, -1), (1, dy, 0), 2))
    pairs.append(((1, -1, 1), (1, 0, 1), 2 * WP))
    pairs.append(((1, 1, 1), None, 0))
    # b2 (3x3, dil 4)
    for dy in range(-1, 2):
        pairs.append(((2, dy, -1), (2, dy, 0), 4))
    pairs.append(((2, -1, 1), (2, 0, 1), 4 * WP))
    pairs.append(((2, 1, 1), None, 0))
    return pairs


PAIRS = _tap_pairs()
NSLOT = 2 * len(PAIRS)          # diag bank slots (46)
BR_PAIRS = {br: [(j, p) for j, p in enumerate(PAIRS) if p[0][0] == br]
            for br in range(3)}
assert [len(BR_PAIRS[b]) for b in range(3)] == [13, 5, 5]


def _tap_off(t, row0):
    """Flat offset (within a slot, before the +GUARD base) of tap t's rhs
    for an output chunk starting at block row `row0`, extended col 0."""
    br, dy, dx = t
    dil = DILS[br]
    return (PAD + row0 + dy * dil) * WP + dx * dil


_PROGRAM_CACHE = {}


def _build_program():
    nc = bacc.Bacc("TRN2", target_bir_lowering=False, debug=False,
                   num_devices=N_CORES)

    x4 = nc.dram_tensor("x4", [SAMPLES_PER_CORE, CIN, PIX], BF16,
                        kind="ExternalInput").ap()
    wconv = nc.dram_tensor("wconv", [128, 512], BF16,
                           kind="ExternalInput").ap()
    wout12_d = nc.dram_tensor("wout12", [128, 3 * 128], BF16,
                              kind="ExternalInput").ap()
    wout1_d = nc.dram_tensor("wout1", [128, COUT], BF16,
                             kind="ExternalInput").ap()
    aT_d = nc.dram_tensor("aT", [128, NSLOT], F32, kind="ExternalInput").ap()
    bT_d = nc.dram_tensor("bT", [128, NSLOT], F32, kind="ExternalInput").ap()
    ident_d = nc.dram_tensor("ident", [128, 128], FP8,
                             kind="ExternalInput").ap()
    convb_d = nc.dram_tensor("convb", [128, 1], F32, kind="ExternalInput").ap()
    biasout_d = nc.dram_tensor("biasout", [128, 3], F32,
                               kind="ExternalInput").ap()
    # per-branch inverse tap scales (folded into branch evac)
    sinv_d = nc.dram_tensor("sinv", [128, 3], F32, kind="ExternalInput").ap()
    y4 = nc.dram_tensor("y4", [SAMPLES_PER_CORE, COUT, PIX], BF16,
                        kind="ExternalOutput").ap()

    with tile.TileContext(nc) as tc:
        with ExitStack() as ctx:
            consts = ctx.enter_context(tc.tile_pool(name="consts", bufs=1))
            xpool = ctx.enter_context(tc.tile_pool(name="xp", bufs=3))
            ffpool = ctx.enter_context(tc.tile_pool(name="ffp", bufs=1))
            dgpool = ctx.enter_context(tc.tile_pool(name="dgp", bufs=1))
            opool = ctx.enter_context(tc.tile_pool(name="op", bufs=2))
            outpool = ctx.enter_context(tc.tile_pool(name="outp", bufs=2))
            smalls = ctx.enter_context(tc.tile_pool(name="sm", bufs=2))
            ps_a = ctx.enter_context(
                tc.tile_pool(name="psa", bufs=1, space="PSUM"))
            ps_out = ctx.enter_context(
                tc.tile_pool(name="pso", bufs=1, space="PSUM"))

            # ---- constants ----
            wconv_t = consts.tile([128, 512], BF16, tag="wconv")
            nc.gpsimd.dma_start(wconv_t[:], wconv[:])
            wout12_t = consts.tile([128, 3 * 128], BF16, tag="wout12")
            nc.gpsimd.dma_start(wout12_t[:], wout12_d[:])
            wout1_t = consts.tile([128, COUT], BF16, tag="wout1")
            nc.gpsimd.dma_start(wout1_t[:], wout1_d[:])
            aT = consts.tile([128, NSLOT], F32, tag="aT")
            nc.gpsimd.dma_start(aT[:], aT_d[:])
            bT = consts.tile([128, NSLOT], F32, tag="bT")
            nc.gpsimd.dma_start(bT[:], bT_d[:])
            ident = consts.tile([128, 128], FP8, tag="ident")
            nc.gpsimd.dma_start(ident[:], ident_d[:])
            convb = consts.tile([128, 1], F32, tag="convb")
            nc.gpsimd.dma_start(convb[:], convb_d[:])
            biasout = consts.tile([128, 3], F32, tag="biasout")
            nc.gpsimd.dma_start(biasout[:], biasout_d[:])
            sinv = consts.tile([128, 3], F32, tag="sinv")
            nc.gpsimd.dma_start(sinv[:], sinv_d[:])

            # persistent FF tiles (one per block parity): 7 slots of padded
            # fp8 f (slot 0) and its shifted copies; pad borders zeroed once.
            ffs, banks = [], []
            zeros = consts.tile([128, PAD * WP], F32, tag="zeros")
            nc.gpsimd.memset(zeros[:], 0.0)
            for par in range(2):
                ff = ffpool.tile([128, NFF * SLP], FP8, tag=f"ff{par}")
                s0 = ff[:, GUARD:GUARD + FSZ]
                v = s0.rearrange("p (r c) -> p r c", c=WP)
                nc.vector.tensor_copy(s0[:, 0:PAD * WP], zeros[:])
                nc.vector.tensor_copy(s0[:, (PAD + H) * WP:FSZ], zeros[:])
                zv = zeros[:].rearrange("p (r c) -> p r c", c=PAD)
                nc.vector.tensor_copy(v[:, PAD:PAD + H, 0:PAD], zv[:, 0:H, :])
                nc.vector.tensor_copy(v[:, PAD:PAD + H, PAD + W:WP],
                                      zv[:, 0:H, :])
                # zero guards + shifted-slot tails once: stale SBUF there can
                # be Inf/NaN bit patterns, and 0 * Inf = NaN even through a
                # zero diag k-tile
                nc.gpsimd.memset(ff[:, 0:GUARD], 0.0)
                nc.gpsimd.memset(ff[:, GUARD + FSZ:SLP], 0.0)
                for si in range(1, NFF):
                    nc.gpsimd.memset(
                        ff[:, si * SLP + SLP - SHIFTS[si]:(si + 1) * SLP],
                        0.0)
                ffs.append(ff)
                bank = dgpool.tile([128, NSLOT, 128], FP8, tag=f"bank{par}")
                nc.gpsimd.memset(bank[:], 0.0)
                banks.append(bank)

            def ff_slot0_rows(ff, r0, nrows):
                """[p, nrows, 64] valid-interior view of slot0 (conv dst)."""
                v = ff[:, GUARD:GUARD + FSZ].rearrange("p (r c) -> p r c",
                                                       c=WP)
                return v[:, PAD + r0:PAD + r0 + nrows, PAD:PAD + W]

            def pair_rhs(ff, t1, shift, row0, ncols):
                """[p, 2, ncols] rhs AP: k-tile1 = slot0 @ off(t1),
                k-tile2 = slot(shift) @ same off (== f @ off+shift)."""
                off = GUARD + _tap_off(t1, row0)
                v = ff[:, off:off + ncols]
                u = v.unsqueeze(1).broadcast_to([128, 2, ncols])
                # singles (shift 0) pair with the zero diag; point k-tile2
                # at slot 1 — small or zero dim-1 strides fault the PE
                slot = SHIFT_SLOT[shift] or 1
                u.ap[1] = [slot * SLP, 2]
                return u

            # round-robin engine pickers for evac/diag work
            def rr(seq):
                i = [0]

                def pick():
                    e = seq[i[0] % len(seq)]
                    i[0] += 1
                    return e
                return pick

            # Pool/GPSIMD cannot read PSUM: psum evacs go to ACT/DVE only;
            # Pool absorbs SBUF-side work (diags, relu) instead.
            diag_eng = rr(["g", "v", "g", "g", "v"])
            evac_eng = rr(["a", "v"])
            out_eng = rr(["a", "v"])

            def emit_conv(blk):
                n0 = 2 * blk
                ff = ffs[blk % 2]
                bank = banks[blk % 2]

                gsums = smalls.tile([128, 8], F32, tag=f"gsums{blk}")
                for q in range(NSLAB):
                    xts = {}
                    for kc in range(4):
                        xt = xpool.tile([128, SLAB], BF16, tag=f"x{kc}")
                        nc.sync.dma_start(
                            xt[:],
                            x4[n0:n0 + 2, kc * 64:(kc + 1) * 64,
                               q * SLAB:(q + 1) * SLAB])
                        xts[kc] = xt
                    for c in range(SLAB // CHUNK):
                        j = q * (SLAB // CHUNK) + c
                        ps = ps_a.tile([128, CHUNK], F32, tag=f"tap{j % 2}")
                        for kc in range(4):
                            nc.tensor.matmul(
                                ps[:],
                                wconv_t[:, kc * 128:(kc + 1) * 128],
                                xts[kc][:, c * CHUNK:(c + 1) * CHUNK],
                                start=(kc == 0), stop=(kc == 3))
                        dst = ff_slot0_rows(ff, 8 * j, 8)
                        nc.scalar.activation(
                            dst, ps[:], AF.Identity,
                            bias=convb[:, 0:1],
                            accum_out=gsums[:, j:j + 1])

                # relu in place over the full slot0 (pads stay 0), split
                # between DVE and Pool
                s0 = ff[:, GUARD:GUARD + FSZ]
                hf = FSZ // 2
                nc.vector.tensor_scalar_max(s0[:, 0:hf], s0[:, 0:hf], 0.0)
                nc.gpsimd.tensor_scalar_max(s0[:, hf:FSZ], s0[:, hf:FSZ], 0.0)

                # shifted copies via SBUF->SBUF DMA; issued from the Pool
                # SWDGE queue so the SP sequencer keeps prefetching x slabs
                for si in range(1, NFF):
                    sh = SHIFTS[si]
                    nc.gpsimd.dma_start(
                        ff[:, si * SLP:si * SLP + SLP - sh],
                        ff[:, sh:SLP])

                # g -> ktile -> diag bank
                gpre = smalls.tile([128, 1], F32, tag=f"gpre{blk}")
                nc.vector.tensor_reduce(gpre[:], gsums[:], op=ALU.add,
                                        axis=mybir.AxisListType.X)
                gt = smalls.tile([128, 1], F32, tag=f"g{blk}")
                nc.scalar.activation(gt[:], gpre[:], AF.Relu,
                                     scale=1.0 / PIX)
                ktile = smalls.tile([128, NSLOT], F32, tag=f"ktile{blk}")
                nc.vector.scalar_tensor_tensor(ktile[:], aT[:], gt[:, 0:1],
                                               bT[:], op0=ALU.mult,
                                               op1=ALU.add)
                nc.vector.tensor_scalar_min(ktile[:], ktile[:], 240.0)
                nc.vector.tensor_scalar_max(ktile[:], ktile[:], -240.0)

                for j, (t1, t2, _sh) in enumerate(PAIRS):
                    for half_, t in ((0, t1), (1, t2)):
                        if t is None:
                            continue
                        sl = 2 * j + half_
                        e = diag_eng()
                        if e == "v":
                            nc.vector.tensor_scalar_mul(
                                bank[:, sl, :], ident[:],
                                ktile[:, sl:sl + 1])
                        elif e == "g":
                            nc.gpsimd.tensor_scalar_mul(
                                bank[:, sl, :], ident[:],
                                ktile[:, sl:sl + 1])
                        else:
                            nc.scalar.activation(
                                bank[:, sl, :], ident[:], AF.Copy,
                                scale=ktile[:, sl:sl + 1])
                return (ff, bank, n0)

            def emit_stage(st, h):
                """One (blk, h): tap chunks and out chunks interleaved so PE
                always has matmul work while ACT/DVE drain psums.
                Interleave: t0 t1 o0 t2 o1 t3 o2 t4 o3 (out chunk oc needs
                branch cols through 512(oc+1), covered by tap chunks
                through ceil(512(oc+1)/448)-1)."""
                ff, bank, n0 = st
                o1t = opool.tile([128, HALF], BF16, tag="o1")
                Xt = opool.tile([128, HALF], BF16, tag="X")
                Yt = opool.tile([128, HALF], BF16, tag="Y")
                pi = [0]
                osbs = {}

                def tap_chunk(ci):
                    lr0, nr = TAPCH[ci]
                    row0 = 32 * h + lr0
                    ncols = nr * WP
                    for br in range(3):
                        ps = ps_a.tile([128, 512], F32,
                                       tag=f"tap{pi[0] % 2}")
                        pi[0] += 1
                        plist = BR_PAIRS[br]
                        for i, (j, (t1, t2, sh)) in enumerate(plist):
                            rhs = pair_rhs(ff, t1, sh, row0, ncols)
                            nc.tensor.matmul(
                                ps[:, 0:ncols], bank[:, 2 * j:2 * j + 2, :],
                                rhs, start=(i == 0),
                                stop=(i == len(plist) - 1),
                                perf_mode=DRMODE)
                        src = ps[:, 0:ncols].rearrange(
                            "p (r c) -> p r c", c=WP)[:, :, PAD:PAD + W]
                        c0 = lr0 * W
                        csl = slice(c0, c0 + nr * W)
                        if br == 0:
                            dsts = [(slice(0, 128), o1t[:, csl])]
                        elif br == 1:
                            dsts = [(slice(0, 64), Xt[0:64, csl]),
                                    (slice(64, 128), Yt[0:64, csl])]
                        else:
                            dsts = [(slice(0, 64), Xt[64:128, csl]),
                                    (slice(64, 128), Yt[64:128, csl])]
                        for psl, dst in dsts:
                            e = evac_eng()
                            sc = sinv[psl, br:br + 1]
                            if e == "a":
                                nc.scalar.activation(dst, src[psl], AF.Copy,
                                                     scale=sc)
                            else:
                                nc.vector.tensor_scalar_mul(dst, src[psl],
                                                            sc)

                def out_chunk(oc):
                    if oc % 2 == 0:
                        for mt in range(3):
                            for s in range(2):
                                osb_tile = outpool.tile(
                                    [128, 2 * CHUNK], BF16,
                                    tag=f"osb{mt}_{s}")
                                osbs[(mt, s)] = osb_tile
                    csl = slice(oc * CHUNK, (oc + 1) * CHUNK)
                    pss = {}
                    for mt in range(3):
                        for s, bt in ((0, Xt), (1, Yt)):
                            ps = ps_out.tile([128, CHUNK], F32,
                                             tag=f"out{s}_{mt}")
                            pss[(s, mt)] = ps
                            nc.tensor.matmul(
                                ps[:],
                                wout12_t[:, mt * 128:(mt + 1) * 128],
                                bt[:, csl], start=True, stop=False)
                    for mt in range(3):
                        for s in range(2):
                            sl = slice(64 * s, 64 * s + 64)
                            nc.tensor.matmul(
                                pss[(s, mt)][:],
                                wout1_t[sl, mt * 128:(mt + 1) * 128],
                                o1t[sl, csl], start=False, stop=True)
                    for mt in range(3):
                        for s in range(2):
                            dst = osbs[(mt, s)][:, (oc % 2) * CHUNK:
                                                (oc % 2 + 1) * CHUNK]
                            if out_eng() == "a":
                                nc.scalar.activation(
                                    dst, pss[(s, mt)][:], AF.Identity,
                                    bias=biasout[:, mt:mt + 1])
                            else:
                                nc.vector.scalar_tensor_tensor(
                                    dst, pss[(s, mt)][:], 1.0,
                                    biasout[:, mt:mt + 1]
                                    .broadcast_to([128, CHUNK]),
                                    op0=ALU.mult, op1=ALU.add)
                    if oc % 2 == 1:
                        g2 = oc // 2
                        px0 = h * HALF + g2 * 2 * CHUNK
                        for mt in range(3):
                            for s in range(2):
                                dst = y4[n0 + s, mt * 128:(mt + 1) * 128,
                                         px0:px0 + 2 * CHUNK]
                                if g2 == 0:
                                    nc.gpsimd.dma_start(dst,
                                                        osbs[(mt, s)][:])
                                else:
                                    nc.scalar.dma_start(dst,
                                                        osbs[(mt, s)][:])

                for kind, i in (("t", 0), ("t", 1), ("o", 0), ("t", 2),
                                ("o", 1), ("t", 3), ("o", 2), ("t", 4),
                                ("o", 3)):
                    if kind == "t":
                        tap_chunk(i)
                    else:
                        out_chunk(i)

            st0 = emit_conv(0)
            emit_stage(st0, 0)
            st1 = emit_conv(1)
            emit_stage(st0, 1)
            emit_stage(st1, 0)
            emit_stage(st1, 1)
    nc.compile()
    return nc


def _get_program():
    if "nc" not in _PROGRAM_CACHE:
        _PROGRAM_CACHE["nc"] = _build_program()
    return _PROGRAM_CACHE["nc"]


def kernel(x, conv_w, conv_b, ck_w, ck_b, ck2_w, ck2_b, ckd4_w, ckd4_b,
           kern_w, kern_b, kern2_w, kern2_b, kernd4_w, kernd4_b,
           fuse_w, fuse_b, fc_w, fc_b):
    import ml_dtypes
    x = np.asarray(x, dtype=np.float32)
    conv_w = np.asarray(conv_w, dtype=np.float32)
    conv_b = np.asarray(conv_b, dtype=np.float32)
    fuse_w = np.asarray(fuse_w, dtype=np.float32)
    fuse_b = np.asarray(fuse_b, dtype=np.float32)
    fc_w = np.asarray(fc_w, dtype=np.float32)
    fc_b = np.asarray(fc_b, dtype=np.float32)

    NB = x.shape[0]
    assert NB == N_CORES * SAMPLES_PER_CORE

    # tap affine coefficients per branch: k_t = a_t * g + b_t
    def fold(sw, sb, kw, kb):
        a = (float(sw) * np.asarray(kw)).astype(np.float32)
        b = (float(sw) * np.asarray(kb) + float(sb)).astype(np.float32)
        return a, b

    a1, b1 = fold(ck_w, ck_b, kern_w, kern_b)        # [25], 5x5 row-major
    a2, b2 = fold(ck2_w, ck2_b, kern2_w, kern2_b)    # [9]
    a3, b3 = fold(ckd4_w, ckd4_b, kernd4_w, kernd4_b)

    def coef(t):
        br, dy, dx = t
        if br == 0:
            return a1[(dy + 2) * 5 + (dx + 2)], b1[(dy + 2) * 5 + (dx + 2)]
        a, b = (a2, b2) if br == 1 else (a3, b3)
        return a[(dy + 1) * 3 + (dx + 1)], b[(dy + 1) * 3 + (dx + 1)]

    # per-branch power-of-2 prescale: bound |k| with g <= GMAX, keep
    # S*|k| <= 200 so fp8e4m3 never saturates
    GMAX = 1.0
    scales = []
    for br in range(3):
        taps = [coef(t1) for (t1, t2, _s) in PAIRS if t1[0] == br]
        taps += [coef(t2) for (t1, t2, _s) in PAIRS
                 if t2 is not None and t2[0] == br]
        bound = max(abs(a) * GMAX + abs(b) for a, b in taps)
        scales.append(2.0 ** np.floor(np.log2(200.0 / max(bound, 1e-30))))
    sinv = np.zeros((128, 3), np.float32)
    for br in range(3):
        sinv[:, br] = 1.0 / scales[br]

    aT = np.zeros((128, NSLOT), np.float32)
    bT = np.zeros((128, NSLOT), np.float32)
    for j, (t1, t2, _sh) in enumerate(PAIRS):
        for half_, t in ((0, t1), (1, t2)):
            if t is None:
                continue
            a, b = coef(t)
            s = scales[t[0]]
            aT[:, 2 * j + half_] = a * s
            bT[:, 2 * j + half_] = b * s

    # folded output weights W_i = fc_w[:, 128i:128(i+1)] @ fuse_w  [384, 64]
    Wi = [fc_w[:, 128 * i:128 * (i + 1)] @ fuse_w for i in range(3)]
    wout12 = np.zeros((128, 3 * 128), dtype=np.float32)
    wout12[0:64, :] = Wi[1].T.reshape(64, COUT)
    wout12[64:128, :] = Wi[2].T.reshape(64, COUT)
    wout12 = wout12.astype(ml_dtypes.bfloat16)
    wout1 = np.zeros((128, COUT), dtype=np.float32)
    wout1[0:64, :] = Wi[0].T
    wout1[64:128, :] = Wi[0].T
    wout1 = wout1.astype(ml_dtypes.bfloat16)
    bias_out = (fc_w @ np.tile(fuse_b, 3) + fc_b).astype(np.float32)
    biasout = bias_out.reshape(3, 128).T.copy()

    wconv = np.zeros((128, 512), dtype=np.float32)
    for kc in range(4):
        wt = conv_w[:, 64 * kc:64 * (kc + 1)].T
        wconv[0:64, 128 * kc:128 * kc + 64] = wt
        wconv[64:128, 128 * kc + 64:128 * (kc + 1)] = wt
    wconv = wconv.astype(ml_dtypes.bfloat16)

    convb = np.concatenate([conv_b, conv_b]).reshape(128, 1).astype(np.float32)
    ident = np.eye(128, dtype=np.float32).astype(ml_dtypes.float8_e4m3)

    nc = _get_program()
    in_maps = []
    xbf = x.reshape(NB, CIN, PIX).astype(ml_dtypes.bfloat16)
    for core in range(N_CORES):
        xs = xbf[core * SAMPLES_PER_CORE:(core + 1) * SAMPLES_PER_CORE]
        in_maps.append({
            "x4": np.ascontiguousarray(xs),
            "wconv": wconv, "wout12": wout12, "wout1": wout1,
            "aT": aT, "bT": bT, "ident": ident, "convb": convb,
            "biasout": biasout, "sinv": sinv,
        })
    res = run_bass_kernel_spmd(nc, in_maps, list(range(N_CORES)))
    out = np.empty((NB, COUT, H, W), dtype=np.float32)
    for core in range(N_CORES):
        out[core * SAMPLES_PER_CORE:(core + 1) * SAMPLES_PER_CORE] = (
            res.results[core]["y4"].reshape(SAMPLES_PER_CORE, COUT, H, W)
            .astype(np.float32))
    return out


# revision 14
# speedup vs baseline: 1.0748x; 1.0472x over previous
"""Trainium2 Bass kernel for nn_DIDAModuleD4 (dynamic depthwise conv module).

Data-parallel over batch: 32 samples -> 8 cores x 4 samples.
Per core, samples are processed in 2 blocks of 2 samples; each block maps the
2x64=128 (sample, channel) pairs onto the 128 SBUF partitions.

Math (per sample, with host-side weight folding):
  f   = relu(conv_w @ x + conv_b)                       [64, 4096]
  g   = relu(mean_px(conv_w @ x + conv_b))              [64]
  k_t = a_t * g + b_t            (43 taps, a/b host-folded scalars)
  o_i = sum_t k_t * shift_t(f)   (depthwise; 5x5, 3x3 d2, 3x3 d4)
  out = sum_i W_i @ o_i + bias_out                      [384, 4096]

All 43 depthwise taps run on the PE as fp8e4 DoubleRow matmuls, two taps per
matmul: f is stored as an fp8 padded tile (72x72 flat domain) plus 6 shifted
copies (SBUF->SBUF DMA, shifts 1/2/4/72/144/288); a tap pair (t, t+delta)
reads k-tiles (slot0@off, slot_delta@off) so the rhs AP is [p, 2, N] with a
large monotonic dim-1 stride (small strides fault the PE).  lhsT k-tiles are
fp8 diag(k_t) matrices built per block from ktile; odd taps pair with an
all-zero diag slot.  DoubleRow costs 0.5 PE cycles/output-column for 2 taps
vs 1.0 for one bf16 tap (4x).  Tap matmuls produce 7-row x 72-col psum
chunks; the 8 pad columns per row are skipped at evacuation (pad wraparound
reads land in neighbor-row pad zeros since |dx*dil| <= PAD).

k values (~2.5e-3) sit in fp8's subnormal range, so ktile is prescaled by a
host-computed power of two per branch and the branch evacuation divides it
back out.  conv (f32r block-diag, 2-sample) and output 1x1s (bf16) are as in
the bf16 baseline; evacuations are spread across ACT/DVE/Pool.
"""

import sys

if "/opt/trn_rl_repo" not in sys.path:
    sys.path.insert(0, "/opt/trn_rl_repo")

import numpy as np
from contextlib import ExitStack

from concourse import bass, mybir, tile, bacc
from concourse.bass_utils import run_bass_kernel_spmd

F32 = mybir.dt.float32
F32R = mybir.dt.float32r
BF16 = mybir.dt.bfloat16
FP8 = mybir.dt.float8e4
AF = mybir.ActivationFunctionType
ALU = mybir.AluOpType
DRMODE = mybir.MatmulPerfMode.DoubleRow

N_CORES = 8
SAMPLES_PER_CORE = 4
CM = 64
CIN = 256
COUT = 384
H = W = 64
PIX = H * W          # 4096
PAD = 4
WP = W + 2 * PAD     # 72
FSZ = WP * WP        # 5184
GUARD = 4            # front/back guard elems per FF slot (OOB garbage ok)
SLP = FSZ + 2 * GUARD  # 5192 slot pitch
SHIFTS = (0, 1, 2, 4, WP, 2 * WP, 4 * WP)   # FF slot shifts
SHIFT_SLOT = {s: i for i, s in enumerate(SHIFTS)}
NFF = len(SHIFTS)    # 7
HALF = 2048          # pixels per half (32 rows)
CHUNK = 512          # conv/out matmul N
SLAB = 1024
NSLAB = PIX // SLAB
# tap-psum chunks per half: 7-row pieces of the 32 rows
TAPCH = ((0, 7), (7, 7), (14, 7), (21, 7), (28, 4))

# taps: (branch, dy, dx); dil = (1, 2, 4)[branch]
DILS = (1, 2, 4)


def _tap_pairs():
    """Pair taps so each pair's flat-offset delta is one of SHIFTS[1:].
    Returns list of (t1, t2_or_None, shift) with t=(br, dy, dx);
    t2's flat offset == t1's + shift (shift==0 for singles)."""
    pairs = []
    # b0 (5x5, dil 1)
    for dy in range(-2, 3):
        pairs.append(((0, dy, -2), (0, dy, -1), 1))
        pairs.append(((0, dy, 0), (0, dy, 1), 1))
    pairs.append(((0, -2, 2), (0, -1, 2), WP))
    pairs.append(((0, 0, 2), (0, 1, 2), WP))
    pairs.append(((0, 2, 2), None, 0))
    # b1 (3x3, dil 2)
    for dy in range(-1, 2):
        pairs.append(((1, dy, -1), (1, dy, 0), 2))
    pairs.append(((1, -1, 1), (1, 0, 1), 2 * WP))
    pairs.append(((1, 1, 1), None, 0))
    # b2 (3x3, dil 4)
    for dy in range(-1, 2):
        pairs.append(((2, dy, -1), (2, dy, 0), 4))
    pairs.append(((2, -1, 1), (2, 0, 1), 4 * WP))
    pairs.append(((2, 1, 1), None, 0))
    return pairs


PAIRS = _tap_pairs()
NSLOT = 2 * len(PAIRS)          # diag bank slots (46)
BR_PAIRS = {br: [(j, p) for j, p in enumerate(PAIRS) if p[0][0] == br]
            for br in range(3)}
assert [len(BR_PAIRS[b]) for b in range(3)] == [13, 5, 5]


def _tap_off(t, row0):
    """Flat offset (within a slot, before the +GUARD base) of tap t's rhs
    for an output chunk starting at block row `row0`, extended col 0."""
    br, dy, dx = t
    dil = DILS[br]
    return (PAD + row0 + dy * dil) * WP + dx * dil


_PROGRAM_CACHE = {}


def _build_program():
    nc = bacc.Bacc("TRN2", target_bir_lowering=False, debug=False,
                   num_devices=N_CORES)

    x4 = nc.dram_tensor("x4", [SAMPLES_PER_CORE, CIN, PIX], BF16,
                        kind="ExternalInput").ap()
    wconv = nc.dram_tensor("wconv", [128, 512], BF16,
                           kind="ExternalInput").ap()
    wout12_d = nc.dram_tensor("wout12", [128, 3 * 128], BF16,
                              kind="ExternalInput").ap()
    wout1_d = nc.dram_tensor("wout1", [128, COUT], BF16,
                             kind="ExternalInput").ap()
    aT_d = nc.dram_tensor("aT", [128, NSLOT], F32, kind="ExternalInput").ap()
    bT_d = nc.dram_tensor("bT", [128, NSLOT], F32, kind="ExternalInput").ap()
    ident_d = nc.dram_tensor("ident", [128, 128], FP8,
                             kind="ExternalInput").ap()
    convb_d = nc.dram_tensor("convb", [128, 1], F32, kind="ExternalInput").ap()
    biasout_d = nc.dram_tensor("biasout", [128, 3], F32,
                               kind="ExternalInput").ap()
    # per-branch inverse tap scales (folded into branch evac)
    sinv_d = nc.dram_tensor("sinv", [128, 3], F32, kind="ExternalInput").ap()
    y4 = nc.dram_tensor("y4", [SAMPLES_PER_CORE, COUT, PIX], BF16,
                        kind="ExternalOutput").ap()

    with tile.TileContext(nc) as tc:
        with ExitStack() as ctx:
            consts = ctx.enter_context(tc.tile_pool(name="consts", bufs=1))
            xpool = ctx.enter_context(tc.tile_pool(name="xp", bufs=3))
            ffpool = ctx.enter_context(tc.tile_pool(name="ffp", bufs=1))
            dgpool = ctx.enter_context(tc.tile_pool(name="dgp", bufs=1))
            opool = ctx.enter_context(tc.tile_pool(name="op", bufs=2))
            outpool = ctx.enter_context(tc.tile_pool(name="outp", bufs=2))
            smalls = ctx.enter_context(tc.tile_pool(name="sm", bufs=2))
            ps_a = ctx.enter_context(
                tc.tile_pool(name="psa", bufs=1, space="PSUM"))
            ps_out = ctx.enter_context(
                tc.tile_pool(name="pso", bufs=1, space="PSUM"))

            # ---- constants ----
            wconv_t = consts.tile([128, 512], BF16, tag="wconv")
            nc.gpsimd.dma_start(wconv_t[:], wconv[:])
            wout12_t = consts.tile([128, 3 * 128], BF16, tag="wout12")
            nc.gpsimd.dma_start(wout12_t[:], wout12_d[:])
            wout1_t = consts.tile([128, COUT], BF16, tag="wout1")
            nc.gpsimd.dma_start(wout1_t[:], wout1_d[:])
            aT = consts.tile([128, NSLOT], F32, tag="aT")
            nc.gpsimd.dma_start(aT[:], aT_d[:])
            bT = consts.tile([128, NSLOT], F32, tag="bT")
            nc.gpsimd.dma_start(bT[:], bT_d[:])
            ident = consts.tile([128, 128], FP8, tag="ident")
            nc.gpsimd.dma_start(ident[:], ident_d[:])
            convb = consts.tile([128, 1], F32, tag="convb")
            nc.gpsimd.dma_start(convb[:], convb_d[:])
            biasout = consts.tile([128, 3], F32, tag="biasout")
            nc.gpsimd.dma_start(biasout[:], biasout_d[:])
            sinv = consts.tile([128, 3], F32, tag="sinv")
            nc.gpsimd.dma_start(sinv[:], sinv_d[:])

            # persistent FF tiles (one per block parity): 7 slots of padded
            # fp8 f (slot 0) and its shifted copies; pad borders zeroed once.
            ffs, banks = [], []
            zeros = consts.tile([128, PAD * WP], F32, tag="zeros")
            nc.gpsimd.memset(zeros[:], 0.0)
            for par in range(2):
                ff = ffpool.tile([128, NFF * SLP], FP8, tag=f"ff{par}")
                s0 = ff[:, GUARD:GUARD + FSZ]
                v = s0.rearrange("p (r c) -> p r c", c=WP)
                nc.vector.tensor_copy(s0[:, 0:PAD * WP], zeros[:])
                nc.vector.tensor_copy(s0[:, (PAD + H) * WP:FSZ], zeros[:])
                zv = zeros[:].rearrange("p (r c) -> p r c", c=PAD)
                nc.vector.tensor_copy(v[:, PAD:PAD + H, 0:PAD], zv[:, 0:H, :])
                nc.vector.tensor_copy(v[:, PAD:PAD + H, PAD + W:WP],
                                      zv[:, 0:H, :])
                # zero guards + shifted-slot tails once: stale SBUF there can
                # be Inf/NaN bit patterns, and 0 * Inf = NaN even through a
                # zero diag k-tile
                nc.gpsimd.memset(ff[:, 0:GUARD], 0.0)
                nc.gpsimd.memset(ff[:, GUARD + FSZ:SLP], 0.0)
                for si in range(1, NFF):
                    nc.gpsimd.memset(
                        ff[:, si * SLP + SLP - SHIFTS[si]:(si + 1) * SLP],
                        0.0)
                ffs.append(ff)
                bank = dgpool.tile([128, NSLOT, 128], FP8, tag=f"bank{par}")
                for j, (t1, t2, _sh) in enumerate(PAIRS):
                    if t2 is None:      # zero partner slot for single taps
                        nc.gpsimd.memset(bank[:, 2 * j + 1, :], 0.0)
                banks.append(bank)

            def ff_slot0_rows(ff, r0, nrows):
                """[p, nrows, 64] valid-interior view of slot0 (conv dst)."""
                v = ff[:, GUARD:GUARD + FSZ].rearrange("p (r c) -> p r c",
                                                       c=WP)
                return v[:, PAD + r0:PAD + r0 + nrows, PAD:PAD + W]

            def pair_rhs(ff, t1, shift, row0, ncols):
                """[p, 2, ncols] rhs AP: k-tile1 = slot0 @ off(t1),
                k-tile2 = slot(shift) @ same off (== f @ off+shift)."""
                off = GUARD + _tap_off(t1, row0)
                v = ff[:, off:off + ncols]
                u = v.unsqueeze(1).broadcast_to([128, 2, ncols])
                # singles (shift 0) pair with the zero diag; point k-tile2
                # at slot 1 — small or zero dim-1 strides fault the PE
                slot = SHIFT_SLOT[shift] or 1
                u.ap[1] = [slot * SLP, 2]
                return u

            # round-robin engine pickers for evac/diag work
            def rr(seq):
                i = [0]

                def pick():
                    e = seq[i[0] % len(seq)]
                    i[0] += 1
                    return e
                return pick

            # Pool/GPSIMD cannot read PSUM: psum evacs go to ACT/DVE only;
            # Pool absorbs SBUF-side work (diags, relu) instead.
            diag_eng = rr(["g", "v", "g", "g", "v"])
            evac_eng = rr(["a", "v"])
            out_eng = rr(["a", "v"])

            def emit_conv(blk):
                n0 = 2 * blk
                ff = ffs[blk % 2]
                bank = banks[blk % 2]

                gsums = smalls.tile([128, 8], F32, tag=f"gsums{blk}")
                for q in range(NSLAB):
                    xts = {}
                    for kc in range(4):
                        xt = xpool.tile([128, SLAB], BF16, tag=f"x{kc}")
                        nc.sync.dma_start(
                            xt[:],
                            x4[n0:n0 + 2, kc * 64:(kc + 1) * 64,
                               q * SLAB:(q + 1) * SLAB])
                        xts[kc] = xt
                    for c in range(SLAB // CHUNK):
                        j = q * (SLAB // CHUNK) + c
                        ps = ps_a.tile([128, CHUNK], F32, tag=f"tap{j % 2}")
                        for kc in range(4):
                            nc.tensor.matmul(
                                ps[:],
                                wconv_t[:, kc * 128:(kc + 1) * 128],
                                xts[kc][:, c * CHUNK:(c + 1) * CHUNK],
                                start=(kc == 0), stop=(kc == 3))
                        dst = ff_slot0_rows(ff, 8 * j, 8)
                        nc.scalar.activation(
                            dst, ps[:], AF.Identity,
                            bias=convb[:, 0:1],
                            accum_out=gsums[:, j:j + 1])

                # relu in place over the full slot0 (pads stay 0), split
                # between DVE and Pool
                s0 = ff[:, GUARD:GUARD + FSZ]
                hf = FSZ // 2
                nc.vector.tensor_scalar_max(s0[:, 0:hf], s0[:, 0:hf], 0.0)
                nc.gpsimd.tensor_scalar_max(s0[:, hf:FSZ], s0[:, hf:FSZ], 0.0)

                # shifted copies via SBUF->SBUF DMA; issued from the Pool
                # SWDGE queue so the SP sequencer keeps prefetching x slabs
                for si in range(1, NFF):
                    sh = SHIFTS[si]
                    nc.sync.dma_start(
                        ff[:, si * SLP:si * SLP + SLP - sh],
                        ff[:, sh:SLP])

                # g -> ktile -> diag bank
                gpre = smalls.tile([128, 1], F32, tag=f"gpre{blk}")
                nc.vector.tensor_reduce(gpre[:], gsums[:], op=ALU.add,
                                        axis=mybir.AxisListType.X)
                gt = smalls.tile([128, 1], F32, tag=f"g{blk}")
                nc.scalar.activation(gt[:], gpre[:], AF.Relu,
                                     scale=1.0 / PIX)
                ktile = smalls.tile([128, NSLOT], F32, tag=f"ktile{blk}")
                nc.vector.scalar_tensor_tensor(ktile[:], aT[:], gt[:, 0:1],
                                               bT[:], op0=ALU.mult,
                                               op1=ALU.add)
                nc.vector.tensor_scalar_min(ktile[:], ktile[:], 240.0)
                nc.vector.tensor_scalar_max(ktile[:], ktile[:], -240.0)

                for j, (t1, t2, _sh) in enumerate(PAIRS):
                    for half_, t in ((0, t1), (1, t2)):
                        if t is None:
                            continue
                        sl = 2 * j + half_
                        e = diag_eng()
                        if e == "v":
                            nc.vector.tensor_scalar_mul(
                                bank[:, sl, :], ident[:],
                                ktile[:, sl:sl + 1])
                        elif e == "g":
                            nc.gpsimd.tensor_scalar_mul(
                                bank[:, sl, :], ident[:],
                                ktile[:, sl:sl + 1])
                        else:
                            nc.scalar.activation(
                                bank[:, sl, :], ident[:], AF.Copy,
                                scale=ktile[:, sl:sl + 1])
                return (ff, bank, n0)

            def emit_stage(st, h):
                """One (blk, h): tap chunks and out chunks interleaved so PE
                always has matmul work while ACT/DVE drain psums.
                Interleave: t0 t1 o0 t2 o1 t3 o2 t4 o3 (out chunk oc needs
                branch cols through 512(oc+1), covered by tap chunks
                through ceil(512(oc+1)/448)-1)."""
                ff, bank, n0 = st
                o1t = opool.tile([128, HALF], BF16, tag="o1")
                Xt = opool.tile([128, HALF], BF16, tag="X")
                Yt = opool.tile([128, HALF], BF16, tag="Y")
                pi = [0]
                osbs = {}

                def tap_chunk(ci):
                    lr0, nr = TAPCH[ci]
                    row0 = 32 * h + lr0
                    ncols = nr * WP
                    for br in range(3):
                        ps = ps_a.tile([128, 512], F32,
                                       tag=f"tap{pi[0] % 2}")
                        pi[0] += 1
                        plist = BR_PAIRS[br]
                        for i, (j, (t1, t2, sh)) in enumerate(plist):
                            rhs = pair_rhs(ff, t1, sh, row0, ncols)
                            nc.tensor.matmul(
                                ps[:, 0:ncols], bank[:, 2 * j:2 * j + 2, :],
                                rhs, start=(i == 0),
                                stop=(i == len(plist) - 1),
                                perf_mode=DRMODE)
                        src = ps[:, 0:ncols].rearrange(
                            "p (r c) -> p r c", c=WP)[:, :, PAD:PAD + W]
                        c0 = lr0 * W
                        csl = slice(c0, c0 + nr * W)
                        if br == 0:
                            dsts = [(slice(0, 128), o1t[:, csl])]
                        elif br == 1:
                            dsts = [(slice(0, 64), Xt[0:64, csl]),
                                    (slice(64, 128), Yt[0:64, csl])]
                        else:
                            dsts = [(slice(0, 64), Xt[64:128, csl]),
                                    (slice(64, 128), Yt[64:128, csl])]
                        for psl, dst in dsts:
                            e = evac_eng()
                            sc = sinv[psl, br:br + 1]
                            if e == "a":
                                nc.scalar.activation(dst, src[psl], AF.Copy,
                                                     scale=sc)
                            else:
                                nc.vector.tensor_scalar_mul(dst, src[psl],
                                                            sc)

                def out_chunk(oc):
                    if oc % 2 == 0:
                        for mt in range(3):
                            for s in range(2):
                                osb_tile = outpool.tile(
                                    [128, 2 * CHUNK], BF16,
                                    tag=f"osb{mt}_{s}")
                                osbs[(mt, s)] = osb_tile
                    csl = slice(oc * CHUNK, (oc + 1) * CHUNK)
                    pss = {}
                    for mt in range(3):
                        for s, bt in ((0, Xt), (1, Yt)):
                            ps = ps_out.tile([128, CHUNK], F32,
                                             tag=f"out{s}_{mt}")
                            pss[(s, mt)] = ps
                            nc.tensor.matmul(
                                ps[:],
                                wout12_t[:, mt * 128:(mt + 1) * 128],
                                bt[:, csl], start=True, stop=False)
                    for mt in range(3):
                        for s in range(2):
                            sl = slice(64 * s, 64 * s + 64)
                            nc.tensor.matmul(
                                pss[(s, mt)][:],
                                wout1_t[sl, mt * 128:(mt + 1) * 128],
                                o1t[sl, csl], start=False, stop=True)
                    for mt in range(3):
                        for s in range(2):
                            dst = osbs[(mt, s)][:, (oc % 2) * CHUNK:
                                                (oc % 2 + 1) * CHUNK]
                            if out_eng() == "a":
                                nc.scalar.activation(
                                    dst, pss[(s, mt)][:], AF.Identity,
                                    bias=biasout[:, mt:mt + 1])
                            else:
                                nc.vector.scalar_tensor_tensor(
                                    dst, pss[(s, mt)][:], 1.0,
                                    biasout[:, mt:mt + 1]
                                    .broadcast_to([128, CHUNK]),
                                    op0=ALU.mult, op1=ALU.add)
                    if oc % 2 == 1:
                        g2 = oc // 2
                        px0 = h * HALF + g2 * 2 * CHUNK
                        for mt in range(3):
                            for s in range(2):
                                dst = y4[n0 + s, mt * 128:(mt + 1) * 128,
                                         px0:px0 + 2 * CHUNK]
                                nc.sync.dma_start(dst, osbs[(mt, s)][:])

                for kind, i in (("t", 0), ("t", 1), ("o", 0), ("t", 2),
                                ("o", 1), ("t", 3), ("o", 2), ("t", 4),
                                ("o", 3)):
                    if kind == "t":
                        tap_chunk(i)
                    else:
                        out_chunk(i)

            st0 = emit_conv(0)
            emit_stage(st0, 0)
            st1 = emit_conv(1)
            emit_stage(st0, 1)
            emit_stage(st1, 0)
            emit_stage(st1, 1)
    nc.compile()
    return nc


def _get_program():
    if "nc" not in _PROGRAM_CACHE:
        _PROGRAM_CACHE["nc"] = _build_program()
    return _PROGRAM_CACHE["nc"]


def kernel(x, conv_w, conv_b, ck_w, ck_b, ck2_w, ck2_b, ckd4_w, ckd4_b,
           kern_w, kern_b, kern2_w, kern2_b, kernd4_w, kernd4_b,
           fuse_w, fuse_b, fc_w, fc_b):
    import ml_dtypes
    x = np.asarray(x, dtype=np.float32)
    conv_w = np.asarray(conv_w, dtype=np.float32)
    conv_b = np.asarray(conv_b, dtype=np.float32)
    fuse_w = np.asarray(fuse_w, dtype=np.float32)
    fuse_b = np.asarray(fuse_b, dtype=np.float32)
    fc_w = np.asarray(fc_w, dtype=np.float32)
    fc_b = np.asarray(fc_b, dtype=np.float32)

    NB = x.shape[0]
    assert NB == N_CORES * SAMPLES_PER_CORE

    # tap affine coefficients per branch: k_t = a_t * g + b_t
    def fold(sw, sb, kw, kb):
        a = (float(sw) * np.asarray(kw)).astype(np.float32)
        b = (float(sw) * np.asarray(kb) + float(sb)).astype(np.float32)
        return a, b

    a1, b1 = fold(ck_w, ck_b, kern_w, kern_b)        # [25], 5x5 row-major
    a2, b2 = fold(ck2_w, ck2_b, kern2_w, kern2_b)    # [9]
    a3, b3 = fold(ckd4_w, ckd4_b, kernd4_w, kernd4_b)

    def coef(t):
        br, dy, dx = t
        if br == 0:
            return a1[(dy + 2) * 5 + (dx + 2)], b1[(dy + 2) * 5 + (dx + 2)]
        a, b = (a2, b2) if br == 1 else (a3, b3)
        return a[(dy + 1) * 3 + (dx + 1)], b[(dy + 1) * 3 + (dx + 1)]

    # per-branch power-of-2 prescale: bound |k| with g <= GMAX, keep
    # S*|k| <= 200 so fp8e4m3 never saturates
    GMAX = 1.0
    scales = []
    for br in range(3):
        taps = [coef(t1) for (t1, t2, _s) in PAIRS if t1[0] == br]
        taps += [coef(t2) for (t1, t2, _s) in PAIRS
                 if t2 is not None and t2[0] == br]
        bound = max(abs(a) * GMAX + abs(b) for a, b in taps)
        scales.append(2.0 ** np.floor(np.log2(200.0 / max(bound, 1e-30))))
    sinv = np.zeros((128, 3), np.float32)
    for br in range(3):
        sinv[:, br] = 1.0 / scales[br]

    aT = np.zeros((128, NSLOT), np.float32)
    bT = np.zeros((128, NSLOT), np.float32)
    for j, (t1, t2, _sh) in enumerate(PAIRS):
        for half_, t in ((0, t1), (1, t2)):
            if t is None:
                continue
            a, b = coef(t)
            s = scales[t[0]]
            aT[:, 2 * j + half_] = a * s
            bT[:, 2 * j + half_] = b * s

    # folded output weights W_i = fc_w[:, 128i:128(i+1)] @ fuse_w  [384, 64]
    Wi = [fc_w[:, 128 * i:128 * (i + 1)] @ fuse_w for i in range(3)]
    wout12 = np.zeros((128, 3 * 128), dtype=np.float32)
    wout12[0:64, :] = Wi[1].T.reshape(64, COUT)
    wout12[64:128, :] = Wi[2].T.reshape(64, COUT)
    wout12 = wout12.astype(ml_dtypes.bfloat16)
    wout1 = np.zeros((128, COUT), dtype=np.float32)
    wout1[0:64, :] = Wi[0].T
    wout1[64:128, :] = Wi[0].T
    wout1 = wout1.astype(ml_dtypes.bfloat16)
    bias_out = (fc_w @ np.tile(fuse_b, 3) + fc_b).astype(np.float32)
    biasout = bias_out.reshape(3, 128).T.copy()

    wconv = np.zeros((128, 512), dtype=np.float32)
    for kc in range(4):
        wt = conv_w[:, 64 * kc:64 * (kc + 1)].T
        wconv[0:64, 128 * kc:128 * kc + 64] = wt
        wconv[64:128, 128 * kc + 64:128 * (kc + 1)] = wt
    wconv = wconv.astype(ml_dtypes.bfloat16)

    convb = np.concatenate([conv_b, conv_b]).reshape(128, 1).astype(np.float32)
    ident = np.eye(128, dtype=np.float32).astype(ml_dtypes.float8_e4m3)

    nc = _get_program()
    in_maps = []
    xbf = x.reshape(NB, CIN, PIX).astype(ml_dtypes.bfloat16)
    for core in range(N_CORES):
        xs = xbf[core * SAMPLES_PER_CORE:(core + 1) * SAMPLES_PER_CORE]
        in_maps.append({
            "x4": np.ascontiguousarray(xs),
            "wconv": wconv, "wout12": wout12, "wout1": wout1,
            "aT": aT, "bT": bT, "ident": ident, "convb": convb,
            "biasout": biasout, "sinv": sinv,
        })
    res = run_bass_kernel_spmd(nc, in_maps, list(range(N_CORES)))
    out = np.empty((NB, COUT, H, W), dtype=np.float32)
    for core in range(N_CORES):
        out[core * SAMPLES_PER_CORE:(core + 1) * SAMPLES_PER_CORE] = (
            res.results[core]["y4"].reshape(SAMPLES_PER_CORE, COUT, H, W)
            .astype(np.float32))
    return out


# revision 16
# speedup vs baseline: 1.0915x; 1.0156x over previous
"""Trainium2 Bass kernel for nn_DIDAModuleD4 (dynamic depthwise conv module).

Data-parallel over batch: 32 samples -> 8 cores x 4 samples.
Per core, samples are processed in 2 blocks of 2 samples; each block maps the
2x64=128 (sample, channel) pairs onto the 128 SBUF partitions.

Math (per sample, with host-side weight folding):
  f   = relu(conv_w @ x + conv_b)                       [64, 4096]
  g   = relu(mean_px(conv_w @ x + conv_b))              [64]
  k_t = a_t * g + b_t            (43 taps, a/b host-folded scalars)
  o_i = sum_t k_t * shift_t(f)   (depthwise; 5x5, 3x3 d2, 3x3 d4)
  out = sum_i W_i @ o_i + bias_out                      [384, 4096]

All 43 depthwise taps run on the PE as fp8e4 DoubleRow matmuls, two taps per
matmul: f is stored as an fp8 padded tile (72x72 flat domain) plus 6 shifted
copies (SBUF->SBUF DMA, shifts 1/2/4/72/144/288); a tap pair (t, t+delta)
reads k-tiles (slot0@off, slot_delta@off) so the rhs AP is [p, 2, N] with a
large monotonic dim-1 stride (small strides fault the PE).  lhsT k-tiles are
fp8 diag(k_t) matrices built per block from ktile; odd taps pair with an
all-zero diag slot.  DoubleRow costs 0.5 PE cycles/output-column for 2 taps
vs 1.0 for one bf16 tap (4x).  Tap matmuls produce 7-row x 72-col psum
chunks; the 8 pad columns per row are skipped at evacuation (pad wraparound
reads land in neighbor-row pad zeros since |dx*dil| <= PAD).

k values (~2.5e-3) sit in fp8's subnormal range, so ktile is prescaled by a
host-computed power of two per branch and the branch evacuation divides it
back out.  conv (f32r block-diag, 2-sample) and output 1x1s (bf16) are as in
the bf16 baseline; evacuations are spread across ACT/DVE/Pool.
"""

import sys

if "/opt/trn_rl_repo" not in sys.path:
    sys.path.insert(0, "/opt/trn_rl_repo")

import numpy as np
from contextlib import ExitStack

from concourse import bass, mybir, tile, bacc
from concourse.bass_utils import run_bass_kernel_spmd

F32 = mybir.dt.float32
F32R = mybir.dt.float32r
BF16 = mybir.dt.bfloat16
FP8 = mybir.dt.float8e4
AF = mybir.ActivationFunctionType
ALU = mybir.AluOpType
DRMODE = mybir.MatmulPerfMode.DoubleRow

N_CORES = 8
SAMPLES_PER_CORE = 4
CM = 64
CIN = 256
COUT = 384
H = W = 64
PIX = H * W          # 4096
PAD = 4
WP = W + 2 * PAD     # 72
FSZ = WP * WP        # 5184
GUARD = 4            # front/back guard elems per FF slot (OOB garbage ok)
SLP = FSZ + 2 * GUARD  # 5192 slot pitch
SHIFTS = (0, 1, 2, 4, WP, 2 * WP, 4 * WP)   # FF slot shifts
SHIFT_SLOT = {s: i for i, s in enumerate(SHIFTS)}
NFF = len(SHIFTS)    # 7
HALF = 2048          # pixels per half (32 rows)
CHUNK = 512          # conv/out matmul N
SLAB = 1024
NSLAB = PIX // SLAB
# tap-psum chunks per half: 7-row pieces of the 32 rows
TAPCH = ((0, 7), (7, 7), (14, 7), (21, 7), (28, 4))

# taps: (branch, dy, dx); dil = (1, 2, 4)[branch]
DILS = (1, 2, 4)


def _tap_pairs():
    """Pair taps so each pair's flat-offset delta is one of SHIFTS[1:].
    Returns list of (t1, t2_or_None, shift) with t=(br, dy, dx);
    t2's flat offset == t1's + shift (shift==0 for singles)."""
    pairs = []
    # b0 (5x5, dil 1)
    for dy in range(-2, 3):
        pairs.append(((0, dy, -2), (0, dy, -1), 1))
        pairs.append(((0, dy, 0), (0, dy, 1), 1))
    pairs.append(((0, -2, 2), (0, -1, 2), WP))
    pairs.append(((0, 0, 2), (0, 1, 2), WP))
    pairs.append(((0, 2, 2), None, 0))
    # b1 (3x3, dil 2)
    for dy in range(-1, 2):
        pairs.append(((1, dy, -1), (1, dy, 0), 2))
    pairs.append(((1, -1, 1), (1, 0, 1), 2 * WP))
    pairs.append(((1, 1, 1), None, 0))
    # b2 (3x3, dil 4)
    for dy in range(-1, 2):
        pairs.append(((2, dy, -1), (2, dy, 0), 4))
    pairs.append(((2, -1, 1), (2, 0, 1), 4 * WP))
    pairs.append(((2, 1, 1), None, 0))
    return pairs


PAIRS = _tap_pairs()
NSLOT = 2 * len(PAIRS)          # diag bank slots (46)
BR_PAIRS = {br: [(j, p) for j, p in enumerate(PAIRS) if p[0][0] == br]
            for br in range(3)}
assert [len(BR_PAIRS[b]) for b in range(3)] == [13, 5, 5]


def _tap_off(t, row0):
    """Flat offset (within a slot, before the +GUARD base) of tap t's rhs
    for an output chunk starting at block row `row0`, extended col 0."""
    br, dy, dx = t
    dil = DILS[br]
    return (PAD + row0 + dy * dil) * WP + dx * dil


_PROGRAM_CACHE = {}


def _build_program():
    nc = bacc.Bacc("TRN2", target_bir_lowering=False, debug=False,
                   num_devices=N_CORES)

    x4 = nc.dram_tensor("x4", [SAMPLES_PER_CORE, CIN, PIX], BF16,
                        kind="ExternalInput").ap()
    wconv = nc.dram_tensor("wconv", [128, 512], BF16,
                           kind="ExternalInput").ap()
    wout12_d = nc.dram_tensor("wout12", [128, 3 * 128], BF16,
                              kind="ExternalInput").ap()
    wout1_d = nc.dram_tensor("wout1", [128, COUT], BF16,
                             kind="ExternalInput").ap()
    aT_d = nc.dram_tensor("aT", [128, NSLOT], F32, kind="ExternalInput").ap()
    bT_d = nc.dram_tensor("bT", [128, NSLOT], F32, kind="ExternalInput").ap()
    ident_d = nc.dram_tensor("ident", [128, 128], FP8,
                             kind="ExternalInput").ap()
    convb_d = nc.dram_tensor("convb", [128, 1], F32, kind="ExternalInput").ap()
    biasout_d = nc.dram_tensor("biasout", [128, 3], F32,
                               kind="ExternalInput").ap()
    # per-branch inverse tap scales (folded into branch evac)
    sinv_d = nc.dram_tensor("sinv", [128, 3], F32, kind="ExternalInput").ap()
    y4 = nc.dram_tensor("y4", [SAMPLES_PER_CORE, COUT, PIX], BF16,
                        kind="ExternalOutput").ap()

    with tile.TileContext(nc) as tc:
        with ExitStack() as ctx:
            consts = ctx.enter_context(tc.tile_pool(name="consts", bufs=1))
            xpool = ctx.enter_context(tc.tile_pool(name="xp", bufs=3))
            ffpool = ctx.enter_context(tc.tile_pool(name="ffp", bufs=1))
            dgpool = ctx.enter_context(tc.tile_pool(name="dgp", bufs=1))
            opool = ctx.enter_context(tc.tile_pool(name="op", bufs=2))
            outpool = ctx.enter_context(tc.tile_pool(name="outp", bufs=2))
            smalls = ctx.enter_context(tc.tile_pool(name="sm", bufs=2))
            ps_a = ctx.enter_context(
                tc.tile_pool(name="psa", bufs=1, space="PSUM"))
            ps_out = ctx.enter_context(
                tc.tile_pool(name="pso", bufs=1, space="PSUM"))

            # ---- constants ----
            wconv_t = consts.tile([128, 512], BF16, tag="wconv")
            nc.gpsimd.dma_start(wconv_t[:], wconv[:])
            wout12_t = consts.tile([128, 3 * 128], BF16, tag="wout12")
            nc.gpsimd.dma_start(wout12_t[:], wout12_d[:])
            wout1_t = consts.tile([128, COUT], BF16, tag="wout1")
            nc.gpsimd.dma_start(wout1_t[:], wout1_d[:])
            aT = consts.tile([128, NSLOT], F32, tag="aT")
            nc.gpsimd.dma_start(aT[:], aT_d[:])
            bT = consts.tile([128, NSLOT], F32, tag="bT")
            nc.gpsimd.dma_start(bT[:], bT_d[:])
            ident = consts.tile([128, 128], FP8, tag="ident")
            nc.gpsimd.dma_start(ident[:], ident_d[:])
            convb = consts.tile([128, 1], F32, tag="convb")
            nc.gpsimd.dma_start(convb[:], convb_d[:])
            biasout = consts.tile([128, 3], F32, tag="biasout")
            nc.gpsimd.dma_start(biasout[:], biasout_d[:])
            sinv = consts.tile([128, 3], F32, tag="sinv")
            nc.gpsimd.dma_start(sinv[:], sinv_d[:])

            # persistent FF tiles (one per block parity): 7 slots of padded
            # fp8 f (slot 0) and its shifted copies; pad borders zeroed once.
            ffs, banks = [], []
            zeros = consts.tile([128, PAD * WP], F32, tag="zeros")
            nc.gpsimd.memset(zeros[:], 0.0)
            for par in range(2):
                ff = ffpool.tile([128, NFF * SLP], FP8, tag=f"ff{par}")
                s0 = ff[:, GUARD:GUARD + FSZ]
                v = s0.rearrange("p (r c) -> p r c", c=WP)
                nc.vector.tensor_copy(s0[:, 0:PAD * WP], zeros[:])
                nc.vector.tensor_copy(s0[:, (PAD + H) * WP:FSZ], zeros[:])
                zv = zeros[:].rearrange("p (r c) -> p r c", c=PAD)
                nc.vector.tensor_copy(v[:, PAD:PAD + H, 0:PAD], zv[:, 0:H, :])
                nc.vector.tensor_copy(v[:, PAD:PAD + H, PAD + W:WP],
                                      zv[:, 0:H, :])
                # zero guards + shifted-slot tails once: stale SBUF there can
                # be Inf/NaN bit patterns, and 0 * Inf = NaN even through a
                # zero diag k-tile
                nc.gpsimd.memset(ff[:, 0:GUARD], 0.0)
                nc.gpsimd.memset(ff[:, GUARD + FSZ:SLP], 0.0)
                for si in range(1, NFF):
                    nc.gpsimd.memset(
                        ff[:, si * SLP + SLP - SHIFTS[si]:(si + 1) * SLP],
                        0.0)
                ffs.append(ff)
                bank = dgpool.tile([128, NSLOT, 128], FP8, tag=f"bank{par}")
                for j, (t1, t2, _sh) in enumerate(PAIRS):
                    if t2 is None:      # zero partner slot for single taps
                        nc.gpsimd.memset(bank[:, 2 * j + 1, :], 0.0)
                banks.append(bank)

            def ff_slot0_rows(ff, r0, nrows):
                """[p, nrows, 64] valid-interior view of slot0 (conv dst)."""
                v = ff[:, GUARD:GUARD + FSZ].rearrange("p (r c) -> p r c",
                                                       c=WP)
                return v[:, PAD + r0:PAD + r0 + nrows, PAD:PAD + W]

            def pair_rhs(ff, t1, shift, row0, ncols):
                """[p, 2, ncols] rhs AP: k-tile1 = slot0 @ off(t1),
                k-tile2 = slot(shift) @ same off (== f @ off+shift)."""
                off = GUARD + _tap_off(t1, row0)
                v = ff[:, off:off + ncols]
                u = v.unsqueeze(1).broadcast_to([128, 2, ncols])
                # singles (shift 0) pair with the zero diag; point k-tile2
                # at slot 1 — small or zero dim-1 strides fault the PE
                slot = SHIFT_SLOT[shift] or 1
                u.ap[1] = [slot * SLP, 2]
                return u

            # round-robin engine pickers for evac/diag work
            def rr(seq):
                i = [0]

                def pick():
                    e = seq[i[0] % len(seq)]
                    i[0] += 1
                    return e
                return pick

            # Pool/GPSIMD cannot read PSUM: psum evacs go to ACT/DVE only;
            # Pool absorbs SBUF-side work (diags, relu) instead.
            diag_eng = rr(["g", "v", "g", "g", "v"])
            evac_eng = rr(["a", "v"])
            out_eng = rr(["a", "v"])

            def emit_conv_loads(blk):
                """Issue all x-slab DMAs for a block (SP queue) so they
                prefetch ahead of the compute that consumes them."""
                n0 = 2 * blk
                slabs = []
                for q in range(NSLAB):
                    xts = {}
                    for kc in range(4):
                        xt = xpool.tile([128, SLAB], BF16,
                                        tag=f"x{blk % 2}_{kc}")
                        nc.sync.dma_start(
                            xt[:],
                            x4[n0:n0 + 2, kc * 64:(kc + 1) * 64,
                               q * SLAB:(q + 1) * SLAB])
                        xts[kc] = xt
                    slabs.append(xts)
                return slabs

            def emit_conv(blk, slabs):
                n0 = 2 * blk
                ff = ffs[blk % 2]
                bank = banks[blk % 2]

                gsums = smalls.tile([128, 8], F32, tag=f"gsums{blk}")
                for q in range(NSLAB):
                    xts = slabs[q]
                    for c in range(SLAB // CHUNK):
                        j = q * (SLAB // CHUNK) + c
                        ps = ps_a.tile([128, CHUNK], F32, tag=f"tap{j % 2}")
                        for kc in range(4):
                            nc.tensor.matmul(
                                ps[:],
                                wconv_t[:, kc * 128:(kc + 1) * 128],
                                xts[kc][:, c * CHUNK:(c + 1) * CHUNK],
                                start=(kc == 0), stop=(kc == 3))
                        dst = ff_slot0_rows(ff, 8 * j, 8)
                        nc.scalar.activation(
                            dst, ps[:], AF.Identity,
                            bias=convb[:, 0:1],
                            accum_out=gsums[:, j:j + 1])

                # relu in place over the full slot0 (pads stay 0), split
                # between DVE and Pool
                s0 = ff[:, GUARD:GUARD + FSZ]
                hf = FSZ // 2
                nc.vector.tensor_scalar_max(s0[:, 0:hf], s0[:, 0:hf], 0.0)
                nc.gpsimd.tensor_scalar_max(s0[:, hf:FSZ], s0[:, hf:FSZ], 0.0)

                # shifted copies via SBUF->SBUF DMA; issued from the Pool
                # SWDGE queue so the SP sequencer keeps prefetching x slabs
                for si in range(1, NFF):
                    sh = SHIFTS[si]
                    nc.gpsimd.dma_start(
                        ff[:, si * SLP:si * SLP + SLP - sh],
                        ff[:, sh:SLP])

                # g -> ktile -> diag bank
                gpre = smalls.tile([128, 1], F32, tag=f"gpre{blk}")
                nc.vector.tensor_reduce(gpre[:], gsums[:], op=ALU.add,
                                        axis=mybir.AxisListType.X)
                gt = smalls.tile([128, 1], F32, tag=f"g{blk}")
                nc.scalar.activation(gt[:], gpre[:], AF.Relu,
                                     scale=1.0 / PIX)
                ktile = smalls.tile([128, NSLOT], F32, tag=f"ktile{blk}")
                nc.vector.scalar_tensor_tensor(ktile[:], aT[:], gt[:, 0:1],
                                               bT[:], op0=ALU.mult,
                                               op1=ALU.add)
                nc.vector.tensor_scalar_min(ktile[:], ktile[:], 240.0)
                nc.vector.tensor_scalar_max(ktile[:], ktile[:], -240.0)

                for j, (t1, t2, _sh) in enumerate(PAIRS):
                    for half_, t in ((0, t1), (1, t2)):
                        if t is None:
                            continue
                        sl = 2 * j + half_
                        e = diag_eng()
                        if e == "v":
                            nc.vector.tensor_scalar_mul(
                                bank[:, sl, :], ident[:],
                                ktile[:, sl:sl + 1])
                        elif e == "g":
                            nc.gpsimd.tensor_scalar_mul(
                                bank[:, sl, :], ident[:],
                                ktile[:, sl:sl + 1])
                        else:
                            nc.scalar.activation(
                                bank[:, sl, :], ident[:], AF.Copy,
                                scale=ktile[:, sl:sl + 1])
                return (ff, bank, n0)

            def emit_stage(st, h):
                """One (blk, h): tap chunks and out chunks interleaved so PE
                always has matmul work while ACT/DVE drain psums.
                Interleave: t0 t1 o0 t2 o1 t3 o2 t4 o3 (out chunk oc needs
                branch cols through 512(oc+1), covered by tap chunks
                through ceil(512(oc+1)/448)-1)."""
                ff, bank, n0 = st
                o1t = opool.tile([128, HALF], BF16, tag="o1")
                Xt = opool.tile([128, HALF], BF16, tag="X")
                Yt = opool.tile([128, HALF], BF16, tag="Y")
                pi = [0]
                osbs = {}

                def tap_chunk(ci):
                    lr0, nr = TAPCH[ci]
                    row0 = 32 * h + lr0
                    ncols = nr * WP
                    for br in range(3):
                        ps = ps_a.tile([128, 512], F32,
                                       tag=f"tap{pi[0] % 2}")
                        pi[0] += 1
                        plist = BR_PAIRS[br]
                        for i, (j, (t1, t2, sh)) in enumerate(plist):
                            rhs = pair_rhs(ff, t1, sh, row0, ncols)
                            nc.tensor.matmul(
                                ps[:, 0:ncols], bank[:, 2 * j:2 * j + 2, :],
                                rhs, start=(i == 0),
                                stop=(i == len(plist) - 1),
                                perf_mode=DRMODE)
                        src = ps[:, 0:ncols].rearrange(
                            "p (r c) -> p r c", c=WP)[:, :, PAD:PAD + W]
                        c0 = lr0 * W
                        csl = slice(c0, c0 + nr * W)
                        if br == 0:
                            dsts = [(slice(0, 128), o1t[:, csl])]
                        elif br == 1:
                            dsts = [(slice(0, 64), Xt[0:64, csl]),
                                    (slice(64, 128), Yt[0:64, csl])]
                        else:
                            dsts = [(slice(0, 64), Xt[64:128, csl]),
                                    (slice(64, 128), Yt[64:128, csl])]
                        for psl, dst in dsts:
                            e = evac_eng()
                            sc = sinv[psl, br:br + 1]
                            if e == "a":
                                nc.scalar.activation(dst, src[psl], AF.Copy,
                                                     scale=sc)
                            else:
                                nc.vector.tensor_scalar_mul(dst, src[psl],
                                                            sc)

                def out_chunk(oc):
                    if oc % 2 == 0:
                        for mt in range(3):
                            for s in range(2):
                                osb_tile = outpool.tile(
                                    [128, 2 * CHUNK], BF16,
                                    tag=f"osb{mt}_{s}")
                                osbs[(mt, s)] = osb_tile
                    csl = slice(oc * CHUNK, (oc + 1) * CHUNK)
                    pss = {}
                    for mt in range(3):
                        for s, bt in ((0, Xt), (1, Yt)):
                            ps = ps_out.tile([128, CHUNK], F32,
                                             tag=f"out{s}_{mt}")
                            pss[(s, mt)] = ps
                            nc.tensor.matmul(
                                ps[:],
                                wout12_t[:, mt * 128:(mt + 1) * 128],
                                bt[:, csl], start=True, stop=False)
                    for mt in range(3):
                        for s in range(2):
                            sl = slice(64 * s, 64 * s + 64)
                            nc.tensor.matmul(
                                pss[(s, mt)][:],
                                wout1_t[sl, mt * 128:(mt + 1) * 128],
                                o1t[sl, csl], start=False, stop=True)
                    for mt in range(3):
                        for s in range(2):
                            dst = osbs[(mt, s)][:, (oc % 2) * CHUNK:
                                                (oc % 2 + 1) * CHUNK]
                            if out_eng() == "a":
                                nc.scalar.activation(
                                    dst, pss[(s, mt)][:], AF.Identity,
                                    bias=biasout[:, mt:mt + 1])
                            else:
                                nc.vector.scalar_tensor_tensor(
                                    dst, pss[(s, mt)][:], 1.0,
                                    biasout[:, mt:mt + 1]
                                    .broadcast_to([128, CHUNK]),
                                    op0=ALU.mult, op1=ALU.add)
                    if oc % 2 == 1:
                        g2 = oc // 2
                        px0 = h * HALF + g2 * 2 * CHUNK
                        for mt in range(3):
                            for s in range(2):
                                dst = y4[n0 + s, mt * 128:(mt + 1) * 128,
                                         px0:px0 + 2 * CHUNK]
                                nc.sync.dma_start(dst, osbs[(mt, s)][:])

                for kind, i in (("t", 0), ("t", 1), ("o", 0), ("t", 2),
                                ("o", 1), ("t", 3), ("o", 2), ("t", 4),
                                ("o", 3)):
                    if kind == "t":
                        tap_chunk(i)
                    else:
                        out_chunk(i)

            sl0 = emit_conv_loads(0)
            st0 = emit_conv(0, sl0)
            sl1 = emit_conv_loads(1)
            emit_stage(st0, 0)
            st1 = emit_conv(1, sl1)
            emit_stage(st0, 1)
            emit_stage(st1, 0)
            emit_stage(st1, 1)
    nc.compile()
    return nc


def _get_program():
    if "nc" not in _PROGRAM_CACHE:
        _PROGRAM_CACHE["nc"] = _build_program()
    return _PROGRAM_CACHE["nc"]


def kernel(x, conv_w, conv_b, ck_w, ck_b, ck2_w, ck2_b, ckd4_w, ckd4_b,
           kern_w, kern_b, kern2_w, kern2_b, kernd4_w, kernd4_b,
           fuse_w, fuse_b, fc_w, fc_b):
    import ml_dtypes
    x = np.asarray(x, dtype=np.float32)
    conv_w = np.asarray(conv_w, dtype=np.float32)
    conv_b = np.asarray(conv_b, dtype=np.float32)
    fuse_w = np.asarray(fuse_w, dtype=np.float32)
    fuse_b = np.asarray(fuse_b, dtype=np.float32)
    fc_w = np.asarray(fc_w, dtype=np.float32)
    fc_b = np.asarray(fc_b, dtype=np.float32)

    NB = x.shape[0]
    assert NB == N_CORES * SAMPLES_PER_CORE

    # tap affine coefficients per branch: k_t = a_t * g + b_t
    def fold(sw, sb, kw, kb):
        a = (float(sw) * np.asarray(kw)).astype(np.float32)
        b = (float(sw) * np.asarray(kb) + float(sb)).astype(np.float32)
        return a, b

    a1, b1 = fold(ck_w, ck_b, kern_w, kern_b)        # [25], 5x5 row-major
    a2, b2 = fold(ck2_w, ck2_b, kern2_w, kern2_b)    # [9]
    a3, b3 = fold(ckd4_w, ckd4_b, kernd4_w, kernd4_b)

    def coef(t):
        br, dy, dx = t
        if br == 0:
            return a1[(dy + 2) * 5 + (dx + 2)], b1[(dy + 2) * 5 + (dx + 2)]
        a, b = (a2, b2) if br == 1 else (a3, b3)
        return a[(dy + 1) * 3 + (dx + 1)], b[(dy + 1) * 3 + (dx + 1)]

    # per-branch power-of-2 prescale: bound |k| with g <= GMAX, keep
    # S*|k| <= 200 so fp8e4m3 never saturates
    GMAX = 1.0
    scales = []
    for br in range(3):
        taps = [coef(t1) for (t1, t2, _s) in PAIRS if t1[0] == br]
        taps += [coef(t2) for (t1, t2, _s) in PAIRS
                 if t2 is not None and t2[0] == br]
        bound = max(abs(a) * GMAX + abs(b) for a, b in taps)
        scales.append(2.0 ** np.floor(np.log2(200.0 / max(bound, 1e-30))))
    sinv = np.zeros((128, 3), np.float32)
    for br in range(3):
        sinv[:, br] = 1.0 / scales[br]

    aT = np.zeros((128, NSLOT), np.float32)
    bT = np.zeros((128, NSLOT), np.float32)
    for j, (t1, t2, _sh) in enumerate(PAIRS):
        for half_, t in ((0, t1), (1, t2)):
            if t is None:
                continue
            a, b = coef(t)
            s = scales[t[0]]
            aT[:, 2 * j + half_] = a * s
            bT[:, 2 * j + half_] = b * s

    # folded output weights W_i = fc_w[:, 128i:128(i+1)] @ fuse_w  [384, 64]
    Wi = [fc_w[:, 128 * i:128 * (i + 1)] @ fuse_w for i in range(3)]
    wout12 = np.zeros((128, 3 * 128), dtype=np.float32)
    wout12[0:64, :] = Wi[1].T.reshape(64, COUT)
    wout12[64:128, :] = Wi[2].T.reshape(64, COUT)
    wout12 = wout12.astype(ml_dtypes.bfloat16)
    wout1 = np.zeros((128, COUT), dtype=np.float32)
    wout1[0:64, :] = Wi[0].T
    wout1[64:128, :] = Wi[0].T
    wout1 = wout1.astype(ml_dtypes.bfloat16)
    bias_out = (fc_w @ np.tile(fuse_b, 3) + fc_b).astype(np.float32)
    biasout = bias_out.reshape(3, 128).T.copy()

    wconv = np.zeros((128, 512), dtype=np.float32)
    for kc in range(4):
        wt = conv_w[:, 64 * kc:64 * (kc + 1)].T
        wconv[0:64, 128 * kc:128 * kc + 64] = wt
        wconv[64:128, 128 * kc + 64:128 * (kc + 1)] = wt
    wconv = wconv.astype(ml_dtypes.bfloat16)

    convb = np.concatenate([conv_b, conv_b]).reshape(128, 1).astype(np.float32)
    ident = np.eye(128, dtype=np.float32).astype(ml_dtypes.float8_e4m3)

    nc = _get_program()
    in_maps = []
    xbf = x.reshape(NB, CIN, PIX).astype(ml_dtypes.bfloat16)
    for core in range(N_CORES):
        xs = xbf[core * SAMPLES_PER_CORE:(core + 1) * SAMPLES_PER_CORE]
        in_maps.append({
            "x4": np.ascontiguousarray(xs),
            "wconv": wconv, "wout12": wout12, "wout1": wout1,
            "aT": aT, "bT": bT, "ident": ident, "convb": convb,
            "biasout": biasout, "sinv": sinv,
        })
    res = run_bass_kernel_spmd(nc, in_maps, list(range(N_CORES)))
    out = np.empty((NB, COUT, H, W), dtype=np.float32)
    for core in range(N_CORES):
        out[core * SAMPLES_PER_CORE:(core + 1) * SAMPLES_PER_CORE] = (
            res.results[core]["y4"].reshape(SAMPLES_PER_CORE, COUT, H, W)
            .astype(np.float32))
    return out


# revision 18
# speedup vs baseline: 1.1369x; 1.0415x over previous
"""Trainium2 Bass kernel for nn_DIDAModuleD4 (dynamic depthwise conv module).

Data-parallel over batch: 32 samples -> 8 cores x 4 samples.
Per core, samples are processed in 2 blocks of 2 samples; each block maps the
2x64=128 (sample, channel) pairs onto the 128 SBUF partitions.

Math (per sample, with host-side weight folding):
  f   = relu(conv_w @ x + conv_b)                       [64, 4096]
  g   = relu(mean_px(conv_w @ x + conv_b))              [64]
  k_t = a_t * g + b_t            (43 taps, a/b host-folded scalars)
  o_i = sum_t k_t * shift_t(f)   (depthwise; 5x5, 3x3 d2, 3x3 d4)
  out = sum_i W_i @ o_i + bias_out                      [384, 4096]

All 43 depthwise taps run on the PE as fp8e4 DoubleRow matmuls, two taps per
matmul: f is stored as an fp8 padded tile (72x72 flat domain) plus 6 shifted
copies (SBUF->SBUF DMA, shifts 1/2/4/72/144/288); a tap pair (t, t+delta)
reads k-tiles (slot0@off, slot_delta@off) so the rhs AP is [p, 2, N] with a
large monotonic dim-1 stride (small strides fault the PE).  lhsT k-tiles are
fp8 diag(k_t) matrices built per block from ktile; odd taps pair with an
all-zero diag slot.  DoubleRow costs 0.5 PE cycles/output-column for 2 taps
vs 1.0 for one bf16 tap (4x).  Tap matmuls produce 7-row x 72-col psum
chunks; the 8 pad columns per row are skipped at evacuation (pad wraparound
reads land in neighbor-row pad zeros since |dx*dil| <= PAD).

k values (~2.5e-3) sit in fp8's subnormal range, so ktile is prescaled by a
host-computed power of two per branch and the branch evacuation divides it
back out.  conv (f32r block-diag, 2-sample) and output 1x1s (bf16) are as in
the bf16 baseline; evacuations are spread across ACT/DVE/Pool.
"""

import sys

if "/opt/trn_rl_repo" not in sys.path:
    sys.path.insert(0, "/opt/trn_rl_repo")

import numpy as np
from contextlib import ExitStack

from concourse import bass, mybir, tile, bacc
from concourse.bass_utils import run_bass_kernel_spmd

F32 = mybir.dt.float32
F32R = mybir.dt.float32r
BF16 = mybir.dt.bfloat16
FP8 = mybir.dt.float8e4
AF = mybir.ActivationFunctionType
ALU = mybir.AluOpType
DRMODE = mybir.MatmulPerfMode.DoubleRow

N_CORES = 8
SAMPLES_PER_CORE = 4
CM = 64
CIN = 256
COUT = 384
H = W = 64
PIX = H * W          # 4096
PAD = 4
WP = W + 2 * PAD     # 72
FSZ = WP * WP        # 5184
GUARD = 4            # front/back guard elems per FF slot (OOB garbage ok)
SLP = FSZ + 2 * GUARD  # 5192 slot pitch
SHIFTS = (0, 1, 2, 4, WP, 2 * WP, 4 * WP)   # FF slot shifts
SHIFT_SLOT = {s: i for i, s in enumerate(SHIFTS)}
NFF = len(SHIFTS)    # 7
HALF = 2048          # pixels per half (32 rows)
CHUNK = 512          # conv/out matmul N
SLAB = 1024
NSLAB = PIX // SLAB
# tap-psum chunks per half: 7-row pieces of the 32 rows
TAPCH = ((0, 7), (7, 7), (14, 7), (21, 7), (28, 4))

# taps: (branch, dy, dx); dil = (1, 2, 4)[branch]
DILS = (1, 2, 4)


def _tap_pairs():
    """Pair taps so each pair's flat-offset delta is one of SHIFTS[1:].
    Returns list of (t1, t2_or_None, shift) with t=(br, dy, dx);
    t2's flat offset == t1's + shift (shift==0 for singles)."""
    pairs = []
    # b0 (5x5, dil 1)
    for dy in range(-2, 3):
        pairs.append(((0, dy, -2), (0, dy, -1), 1))
        pairs.append(((0, dy, 0), (0, dy, 1), 1))
    pairs.append(((0, -2, 2), (0, -1, 2), WP))
    pairs.append(((0, 0, 2), (0, 1, 2), WP))
    pairs.append(((0, 2, 2), None, 0))
    # b1 (3x3, dil 2)
    for dy in range(-1, 2):
        pairs.append(((1, dy, -1), (1, dy, 0), 2))
    pairs.append(((1, -1, 1), (1, 0, 1), 2 * WP))
    pairs.append(((1, 1, 1), None, 0))
    # b2 (3x3, dil 4)
    for dy in range(-1, 2):
        pairs.append(((2, dy, -1), (2, dy, 0), 4))
    pairs.append(((2, -1, 1), (2, 0, 1), 4 * WP))
    pairs.append(((2, 1, 1), None, 0))
    return pairs


PAIRS = _tap_pairs()
NSLOT = 2 * len(PAIRS)          # diag bank slots (46)
BR_PAIRS = {br: [(j, p) for j, p in enumerate(PAIRS) if p[0][0] == br]
            for br in range(3)}
assert [len(BR_PAIRS[b]) for b in range(3)] == [13, 5, 5]


def _tap_off(t, row0):
    """Flat offset (within a slot, before the +GUARD base) of tap t's rhs
    for an output chunk starting at block row `row0`, extended col 0."""
    br, dy, dx = t
    dil = DILS[br]
    return (PAD + row0 + dy * dil) * WP + dx * dil


_PROGRAM_CACHE = {}


def _build_program():
    nc = bacc.Bacc("TRN2", target_bir_lowering=False, debug=False,
                   num_devices=N_CORES)

    x4 = nc.dram_tensor("x4", [SAMPLES_PER_CORE, CIN, PIX], BF16,
                        kind="ExternalInput").ap()
    wconv = nc.dram_tensor("wconv", [128, 512], BF16,
                           kind="ExternalInput").ap()
    wout12_d = nc.dram_tensor("wout12", [128, 3 * 128], BF16,
                              kind="ExternalInput").ap()
    wout1_d = nc.dram_tensor("wout1", [128, COUT], BF16,
                             kind="ExternalInput").ap()
    aT_d = nc.dram_tensor("aT", [128, NSLOT], F32, kind="ExternalInput").ap()
    bT_d = nc.dram_tensor("bT", [128, NSLOT], F32, kind="ExternalInput").ap()
    ident_d = nc.dram_tensor("ident", [128, 128], FP8,
                             kind="ExternalInput").ap()
    convb_d = nc.dram_tensor("convb", [128, 1], F32, kind="ExternalInput").ap()
    biasout_d = nc.dram_tensor("biasout", [128, 3], F32,
                               kind="ExternalInput").ap()
    # per-branch inverse tap scales (folded into branch evac)
    sinv_d = nc.dram_tensor("sinv", [128, 3], F32, kind="ExternalInput").ap()
    y4 = nc.dram_tensor("y4", [SAMPLES_PER_CORE, COUT, PIX], BF16,
                        kind="ExternalOutput").ap()

    with tile.TileContext(nc) as tc:
        with ExitStack() as ctx:
            consts = ctx.enter_context(tc.tile_pool(name="consts", bufs=1))
            xpool = ctx.enter_context(tc.tile_pool(name="xp", bufs=3))
            ffpool = ctx.enter_context(tc.tile_pool(name="ffp", bufs=1))
            dgpool = ctx.enter_context(tc.tile_pool(name="dgp", bufs=1))
            opool = ctx.enter_context(tc.tile_pool(name="op", bufs=2))
            outpool = ctx.enter_context(tc.tile_pool(name="outp", bufs=2))
            smalls = ctx.enter_context(tc.tile_pool(name="sm", bufs=2))
            ps_a = ctx.enter_context(
                tc.tile_pool(name="psa", bufs=1, space="PSUM"))
            ps_out = ctx.enter_context(
                tc.tile_pool(name="pso", bufs=1, space="PSUM"))

            # ---- constants (conv-critical ones first, on the scalar
            # queue; ACT table warmed immediately so the first conv evac
            # does not eat the 1.3us table load) ----
            warm = consts.tile([128, 1], F32, tag="warm")
            nc.gpsimd.memset(warm[:], 0.0)
            nc.scalar.activation(warm[:], warm[:], AF.Identity)
            wconv_t = consts.tile([128, 512], BF16, tag="wconv")
            nc.scalar.dma_start(wconv_t[:], wconv[:])
            convb = consts.tile([128, 1], F32, tag="convb")
            nc.scalar.dma_start(convb[:], convb_d[:])
            wout12_t = consts.tile([128, 3 * 128], BF16, tag="wout12")
            nc.gpsimd.dma_start(wout12_t[:], wout12_d[:])
            wout1_t = consts.tile([128, COUT], BF16, tag="wout1")
            nc.gpsimd.dma_start(wout1_t[:], wout1_d[:])
            aT = consts.tile([128, NSLOT], F32, tag="aT")
            nc.gpsimd.dma_start(aT[:], aT_d[:])
            bT = consts.tile([128, NSLOT], F32, tag="bT")
            nc.gpsimd.dma_start(bT[:], bT_d[:])
            ident = consts.tile([128, 128], FP8, tag="ident")
            nc.gpsimd.dma_start(ident[:], ident_d[:])
            biasout = consts.tile([128, 3], F32, tag="biasout")
            nc.gpsimd.dma_start(biasout[:], biasout_d[:])
            sinv = consts.tile([128, 3], F32, tag="sinv")
            nc.gpsimd.dma_start(sinv[:], sinv_d[:])

            # persistent FF tiles (one per block parity): 7 slots of padded
            # fp8 f (slot 0) and its shifted copies; pad borders zeroed once.
            ffs, banks = [], []
            zeros = consts.tile([128, PAD * WP], F32, tag="zeros")
            nc.gpsimd.memset(zeros[:], 0.0)
            for par in range(2):
                ff = ffpool.tile([128, NFF * SLP], FP8, tag=f"ff{par}")
                s0 = ff[:, GUARD:GUARD + FSZ]
                v = s0.rearrange("p (r c) -> p r c", c=WP)
                nc.vector.tensor_copy(s0[:, 0:PAD * WP], zeros[:])
                nc.vector.tensor_copy(s0[:, (PAD + H) * WP:FSZ], zeros[:])
                zv = zeros[:].rearrange("p (r c) -> p r c", c=PAD)
                nc.vector.tensor_copy(v[:, PAD:PAD + H, 0:PAD], zv[:, 0:H, :])
                nc.vector.tensor_copy(v[:, PAD:PAD + H, PAD + W:WP],
                                      zv[:, 0:H, :])
                # zero guards + shifted-slot tails once: stale SBUF there can
                # be Inf/NaN bit patterns, and 0 * Inf = NaN even through a
                # zero diag k-tile
                nc.gpsimd.memset(ff[:, 0:GUARD], 0.0)
                nc.gpsimd.memset(ff[:, GUARD + FSZ:SLP], 0.0)
                for si in range(1, NFF):
                    nc.gpsimd.memset(
                        ff[:, si * SLP + SLP - SHIFTS[si]:(si + 1) * SLP],
                        0.0)
                ffs.append(ff)
                bank = dgpool.tile([128, NSLOT, 128], FP8, tag=f"bank{par}")
                for j, (t1, t2, _sh) in enumerate(PAIRS):
                    if t2 is None:      # zero partner slot for single taps
                        nc.gpsimd.memset(bank[:, 2 * j + 1, :], 0.0)
                banks.append(bank)

            def ff_slot0_rows(ff, r0, nrows):
                """[p, nrows, 64] valid-interior view of slot0 (conv dst)."""
                v = ff[:, GUARD:GUARD + FSZ].rearrange("p (r c) -> p r c",
                                                       c=WP)
                return v[:, PAD + r0:PAD + r0 + nrows, PAD:PAD + W]

            def pair_rhs(ff, t1, shift, row0, ncols):
                """[p, 2, ncols] rhs AP: k-tile1 = slot0 @ off(t1),
                k-tile2 = slot(shift) @ same off (== f @ off+shift)."""
                off = GUARD + _tap_off(t1, row0)
                v = ff[:, off:off + ncols]
                u = v.unsqueeze(1).broadcast_to([128, 2, ncols])
                # singles (shift 0) pair with the zero diag; point k-tile2
                # at slot 1 — small or zero dim-1 strides fault the PE
                slot = SHIFT_SLOT[shift] or 1
                u.ap[1] = [slot * SLP, 2]
                return u

            # round-robin engine pickers for evac/diag work
            def rr(seq):
                i = [0]

                def pick():
                    e = seq[i[0] % len(seq)]
                    i[0] += 1
                    return e
                return pick

            # Pool/GPSIMD cannot read PSUM: psum evacs go to ACT/DVE only;
            # Pool absorbs SBUF-side work (diags, relu) instead.
            diag_eng = rr(["g", "v", "g", "g", "v"])
            evac_eng = rr(["a", "v"])
            out_eng = rr(["a", "v"])

            def emit_conv_loads(blk):
                """Issue all x-slab DMAs for a block (SP queue) so they
                prefetch ahead of the compute that consumes them."""
                n0 = 2 * blk
                slabs = []
                for q in range(NSLAB):
                    xts = {}
                    for kc in range(4):
                        xt = xpool.tile([128, SLAB], BF16,
                                        tag=f"x{blk % 2}_{kc}")
                        nc.sync.dma_start(
                            xt[:],
                            x4[n0:n0 + 2, kc * 64:(kc + 1) * 64,
                               q * SLAB:(q + 1) * SLAB])
                        xts[kc] = xt
                    slabs.append(xts)
                return slabs

            def emit_conv(blk, slabs):
                n0 = 2 * blk
                ff = ffs[blk % 2]
                bank = banks[blk % 2]

                gsums = smalls.tile([128, 8], F32, tag=f"gsums{blk}")
                for q in range(NSLAB):
                    xts = slabs[q]
                    for c in range(SLAB // CHUNK):
                        j = q * (SLAB // CHUNK) + c
                        ps = ps_a.tile([128, CHUNK], F32, tag=f"tap{j % 2}")
                        for kc in range(4):
                            nc.tensor.matmul(
                                ps[:],
                                wconv_t[:, kc * 128:(kc + 1) * 128],
                                xts[kc][:, c * CHUNK:(c + 1) * CHUNK],
                                start=(kc == 0), stop=(kc == 3))
                        dst = ff_slot0_rows(ff, 8 * j, 8)
                        nc.scalar.activation(
                            dst, ps[:], AF.Identity,
                            bias=convb[:, 0:1],
                            accum_out=gsums[:, j:j + 1])

                # relu in place over the full slot0 (pads stay 0), split
                # between DVE and Pool
                s0 = ff[:, GUARD:GUARD + FSZ]
                hf = FSZ // 2
                nc.vector.tensor_scalar_max(s0[:, 0:hf], s0[:, 0:hf], 0.0)
                nc.gpsimd.tensor_scalar_max(s0[:, hf:FSZ], s0[:, hf:FSZ], 0.0)

                # shifted copies via SBUF->SBUF DMA; issued from the Pool
                # SWDGE queue so the SP sequencer keeps prefetching x slabs
                cp_eng = [nc.sync, nc.scalar, nc.gpsimd]
                for si in range(1, NFF):
                    sh = SHIFTS[si]
                    cp_eng[(si - 1) % 3].dma_start(
                        ff[:, si * SLP:si * SLP + SLP - sh],
                        ff[:, sh:SLP])

                # g -> ktile -> diag bank
                gpre = smalls.tile([128, 1], F32, tag=f"gpre{blk}")
                nc.vector.tensor_reduce(gpre[:], gsums[:], op=ALU.add,
                                        axis=mybir.AxisListType.X)
                gt = smalls.tile([128, 1], F32, tag=f"g{blk}")
                nc.scalar.activation(gt[:], gpre[:], AF.Relu,
                                     scale=1.0 / PIX)
                ktile = smalls.tile([128, NSLOT], F32, tag=f"ktile{blk}")
                nc.vector.scalar_tensor_tensor(ktile[:], aT[:], gt[:, 0:1],
                                               bT[:], op0=ALU.mult,
                                               op1=ALU.add)
                nc.vector.tensor_scalar_min(ktile[:], ktile[:], 240.0)
                nc.vector.tensor_scalar_max(ktile[:], ktile[:], -240.0)

                for j, (t1, t2, _sh) in enumerate(PAIRS):
                    for half_, t in ((0, t1), (1, t2)):
                        if t is None:
                            continue
                        sl = 2 * j + half_
                        e = diag_eng()
                        if e == "v":
                            nc.vector.tensor_scalar_mul(
                                bank[:, sl, :], ident[:],
                                ktile[:, sl:sl + 1])
                        elif e == "g":
                            nc.gpsimd.tensor_scalar_mul(
                                bank[:, sl, :], ident[:],
                                ktile[:, sl:sl + 1])
                        else:
                            nc.scalar.activation(
                                bank[:, sl, :], ident[:], AF.Copy,
                                scale=ktile[:, sl:sl + 1])
                return (ff, bank, n0)

            def emit_stage(st, h):
                """One (blk, h): tap chunks and out chunks interleaved so PE
                always has matmul work while ACT/DVE drain psums.
                Interleave: t0 t1 o0 t2 o1 t3 o2 t4 o3 (out chunk oc needs
                branch cols through 512(oc+1), covered by tap chunks
                through ceil(512(oc+1)/448)-1)."""
                ff, bank, n0 = st
                o1t = opool.tile([128, HALF], BF16, tag="o1")
                Xt = opool.tile([128, HALF], BF16, tag="X")
                Yt = opool.tile([128, HALF], BF16, tag="Y")
                pi = [0]
                osbs = {}

                def tap_chunk(ci):
                    lr0, nr = TAPCH[ci]
                    row0 = 32 * h + lr0
                    ncols = nr * WP
                    for br in range(3):
                        ps = ps_a.tile([128, 512], F32,
                                       tag=f"tap{pi[0] % 2}")
                        pi[0] += 1
                        plist = BR_PAIRS[br]
                        for i, (j, (t1, t2, sh)) in enumerate(plist):
                            rhs = pair_rhs(ff, t1, sh, row0, ncols)
                            nc.tensor.matmul(
                                ps[:, 0:ncols], bank[:, 2 * j:2 * j + 2, :],
                                rhs, start=(i == 0),
                                stop=(i == len(plist) - 1),
                                perf_mode=DRMODE)
                        src = ps[:, 0:ncols].rearrange(
                            "p (r c) -> p r c", c=WP)[:, :, PAD:PAD + W]
                        c0 = lr0 * W
                        csl = slice(c0, c0 + nr * W)
                        if br == 0:
                            dsts = [(slice(0, 128), o1t[:, csl])]
                        elif br == 1:
                            dsts = [(slice(0, 64), Xt[0:64, csl]),
                                    (slice(64, 128), Yt[0:64, csl])]
                        else:
                            dsts = [(slice(0, 64), Xt[64:128, csl]),
                                    (slice(64, 128), Yt[64:128, csl])]
                        for psl, dst in dsts:
                            e = evac_eng()
                            sc = sinv[psl, br:br + 1]
                            if e == "a":
                                nc.scalar.activation(dst, src[psl], AF.Copy,
                                                     scale=sc)
                            else:
                                nc.vector.tensor_scalar_mul(dst, src[psl],
                                                            sc)

                def out_chunk(oc):
                    if oc % 2 == 0:
                        for mt in range(3):
                            for s in range(2):
                                osb_tile = outpool.tile(
                                    [128, 2 * CHUNK], BF16,
                                    tag=f"osb{mt}_{s}")
                                osbs[(mt, s)] = osb_tile
                    csl = slice(oc * CHUNK, (oc + 1) * CHUNK)
                    pss = {}
                    for mt in range(3):
                        for s, bt in ((0, Xt), (1, Yt)):
                            ps = ps_out.tile([128, CHUNK], F32,
                                             tag=f"out{s}_{mt}")
                            pss[(s, mt)] = ps
                            nc.tensor.matmul(
                                ps[:],
                                wout12_t[:, mt * 128:(mt + 1) * 128],
                                bt[:, csl], start=True, stop=False)
                    for mt in range(3):
                        for s in range(2):
                            sl = slice(64 * s, 64 * s + 64)
                            nc.tensor.matmul(
                                pss[(s, mt)][:],
                                wout1_t[sl, mt * 128:(mt + 1) * 128],
                                o1t[sl, csl], start=False, stop=True)
                    for mt in range(3):
                        for s in range(2):
                            dst = osbs[(mt, s)][:, (oc % 2) * CHUNK:
                                                (oc % 2 + 1) * CHUNK]
                            if out_eng() == "a":
                                nc.scalar.activation(
                                    dst, pss[(s, mt)][:], AF.Identity,
                                    bias=biasout[:, mt:mt + 1])
                            else:
                                nc.vector.scalar_tensor_tensor(
                                    dst, pss[(s, mt)][:], 1.0,
                                    biasout[:, mt:mt + 1]
                                    .broadcast_to([128, CHUNK]),
                                    op0=ALU.mult, op1=ALU.add)
                    px0 = h * HALF + oc * CHUNK
                    half_sl = slice((oc % 2) * CHUNK, (oc % 2 + 1) * CHUNK)
                    for mt in range(3):
                        for s in range(2):
                            dst = y4[n0 + s, mt * 128:(mt + 1) * 128,
                                     px0:px0 + CHUNK]
                            nc.sync.dma_start(dst, osbs[(mt, s)][:, half_sl])

                for kind, i in (("t", 0), ("t", 1), ("o", 0), ("t", 2),
                                ("o", 1), ("t", 3), ("o", 2), ("t", 4),
                                ("o", 3)):
                    if kind == "t":
                        tap_chunk(i)
                    else:
                        out_chunk(i)

            sl0 = emit_conv_loads(0)
            st0 = emit_conv(0, sl0)
            sl1 = emit_conv_loads(1)
            emit_stage(st0, 0)
            st1 = emit_conv(1, sl1)
            emit_stage(st0, 1)
            emit_stage(st1, 0)
            emit_stage(st1, 1)
    nc.compile()
    return nc


def _get_program():
    if "nc" not in _PROGRAM_CACHE:
        _PROGRAM_CACHE["nc"] = _build_program()
    return _PROGRAM_CACHE["nc"]


def kernel(x, conv_w, conv_b, ck_w, ck_b, ck2_w, ck2_b, ckd4_w, ckd4_b,
           kern_w, kern_b, kern2_w, kern2_b, kernd4_w, kernd4_b,
           fuse_w, fuse_b, fc_w, fc_b):
    import ml_dtypes
    x = np.asarray(x, dtype=np.float32)
    conv_w = np.asarray(conv_w, dtype=np.float32)
    conv_b = np.asarray(conv_b, dtype=np.float32)
    fuse_w = np.asarray(fuse_w, dtype=np.float32)
    fuse_b = np.asarray(fuse_b, dtype=np.float32)
    fc_w = np.asarray(fc_w, dtype=np.float32)
    fc_b = np.asarray(fc_b, dtype=np.float32)

    NB = x.shape[0]
    assert NB == N_CORES * SAMPLES_PER_CORE

    # tap affine coefficients per branch: k_t = a_t * g + b_t
    def fold(sw, sb, kw, kb):
        a = (float(sw) * np.asarray(kw)).astype(np.float32)
        b = (float(sw) * np.asarray(kb) + float(sb)).astype(np.float32)
        return a, b

    a1, b1 = fold(ck_w, ck_b, kern_w, kern_b)        # [25], 5x5 row-major
    a2, b2 = fold(ck2_w, ck2_b, kern2_w, kern2_b)    # [9]
    a3, b3 = fold(ckd4_w, ckd4_b, kernd4_w, kernd4_b)

    def coef(t):
        br, dy, dx = t
        if br == 0:
            return a1[(dy + 2) * 5 + (dx + 2)], b1[(dy + 2) * 5 + (dx + 2)]
        a, b = (a2, b2) if br == 1 else (a3, b3)
        return a[(dy + 1) * 3 + (dx + 1)], b[(dy + 1) * 3 + (dx + 1)]

    # per-branch power-of-2 prescale: bound |k| with g <= GMAX, keep
    # S*|k| <= 200 so fp8e4m3 never saturates
    GMAX = 1.0
    scales = []
    for br in range(3):
        taps = [coef(t1) for (t1, t2, _s) in PAIRS if t1[0] == br]
        taps += [coef(t2) for (t1, t2, _s) in PAIRS
                 if t2 is not None and t2[0] == br]
        bound = max(abs(a) * GMAX + abs(b) for a, b in taps)
        scales.append(2.0 ** np.floor(np.log2(200.0 / max(bound, 1e-30))))
    sinv = np.zeros((128, 3), np.float32)
    for br in range(3):
        sinv[:, br] = 1.0 / scales[br]

    aT = np.zeros((128, NSLOT), np.float32)
    bT = np.zeros((128, NSLOT), np.float32)
    for j, (t1, t2, _sh) in enumerate(PAIRS):
        for half_, t in ((0, t1), (1, t2)):
            if t is None:
                continue
            a, b = coef(t)
            s = scales[t[0]]
            aT[:, 2 * j + half_] = a * s
            bT[:, 2 * j + half_] = b * s

    # folded output weights W_i = fc_w[:, 128i:128(i+1)] @ fuse_w  [384, 64]
    Wi = [fc_w[:, 128 * i:128 * (i + 1)] @ fuse_w for i in range(3)]
    wout12 = np.zeros((128, 3 * 128), dtype=np.float32)
    wout12[0:64, :] = Wi[1].T.reshape(64, COUT)
    wout12[64:128, :] = Wi[2].T.reshape(64, COUT)
    wout12 = wout12.astype(ml_dtypes.bfloat16)
    wout1 = np.zeros((128, COUT), dtype=np.float32)
    wout1[0:64, :] = Wi[0].T
    wout1[64:128, :] = Wi[0].T
    wout1 = wout1.astype(ml_dtypes.bfloat16)
    bias_out = (fc_w @ np.tile(fuse_b, 3) + fc_b).astype(np.float32)
    biasout = bias_out.reshape(3, 128).T.copy()

    wconv = np.zeros((128, 512), dtype=np.float32)
    for kc in range(4):
        wt = conv_w[:, 64 * kc:64 * (kc + 1)].T
        wconv[0:64, 128 * kc:128 * kc + 64] = wt
        wconv[64:128, 128 * kc + 64:128 * (kc + 1)] = wt
    wconv = wconv.astype(ml_dtypes.bfloat16)

    convb = np.concatenate([conv_b, conv_b]).reshape(128, 1).astype(np.float32)
    ident = np.eye(128, dtype=np.float32).astype(ml_dtypes.float8_e4m3)

    nc = _get_program()
    in_maps = []
    xbf = x.reshape(NB, CIN, PIX).astype(ml_dtypes.bfloat16)
    for core in range(N_CORES):
        xs = xbf[core * SAMPLES_PER_CORE:(core + 1) * SAMPLES_PER_CORE]
        in_maps.append({
            "x4": np.ascontiguousarray(xs),
            "wconv": wconv, "wout12": wout12, "wout1": wout1,
            "aT": aT, "bT": bT, "ident": ident, "convb": convb,
            "biasout": biasout, "sinv": sinv,
        })
    res = run_bass_kernel_spmd(nc, in_maps, list(range(N_CORES)))
    out = np.empty((NB, COUT, H, W), dtype=np.float32)
    for core in range(N_CORES):
        out[core * SAMPLES_PER_CORE:(core + 1) * SAMPLES_PER_CORE] = (
            res.results[core]["y4"].reshape(SAMPLES_PER_CORE, COUT, H, W)
            .astype(np.float32))
    return out


# revision 19
# speedup vs baseline: 1.1514x; 1.0128x over previous
"""Trainium2 Bass kernel for nn_DIDAModuleD4 (dynamic depthwise conv module).

Data-parallel over batch: 32 samples -> 8 cores x 4 samples.
Per core, samples are processed in 2 blocks of 2 samples; each block maps the
2x64=128 (sample, channel) pairs onto the 128 SBUF partitions.

Math (per sample, with host-side weight folding):
  f   = relu(conv_w @ x + conv_b)                       [64, 4096]
  g   = relu(mean_px(conv_w @ x + conv_b))              [64]
  k_t = a_t * g + b_t            (43 taps, a/b host-folded scalars)
  o_i = sum_t k_t * shift_t(f)   (depthwise; 5x5, 3x3 d2, 3x3 d4)
  out = sum_i W_i @ o_i + bias_out                      [384, 4096]

All 43 depthwise taps run on the PE as fp8e4 DoubleRow matmuls, two taps per
matmul: f is stored as an fp8 padded tile (72x72 flat domain) plus 6 shifted
copies (SBUF->SBUF DMA, shifts 1/2/4/72/144/288); a tap pair (t, t+delta)
reads k-tiles (slot0@off, slot_delta@off) so the rhs AP is [p, 2, N] with a
large monotonic dim-1 stride (small strides fault the PE).  lhsT k-tiles are
fp8 diag(k_t) matrices built per block from ktile; odd taps pair with an
all-zero diag slot.  DoubleRow costs 0.5 PE cycles/output-column for 2 taps
vs 1.0 for one bf16 tap (4x).  Tap matmuls produce 7-row x 72-col psum
chunks; the 8 pad columns per row are skipped at evacuation (pad wraparound
reads land in neighbor-row pad zeros since |dx*dil| <= PAD).

k values (~2.5e-3) sit in fp8's subnormal range, so ktile is prescaled by a
host-computed power of two per branch and the branch evacuation divides it
back out.  conv (f32r block-diag, 2-sample) and output 1x1s (bf16) are as in
the bf16 baseline; evacuations are spread across ACT/DVE/Pool.
"""

import sys

if "/opt/trn_rl_repo" not in sys.path:
    sys.path.insert(0, "/opt/trn_rl_repo")

import numpy as np
from contextlib import ExitStack

from concourse import bass, mybir, tile, bacc
from concourse.bass_utils import run_bass_kernel_spmd

F32 = mybir.dt.float32
F32R = mybir.dt.float32r
BF16 = mybir.dt.bfloat16
FP8 = mybir.dt.float8e4
AF = mybir.ActivationFunctionType
ALU = mybir.AluOpType
DRMODE = mybir.MatmulPerfMode.DoubleRow

N_CORES = 8
SAMPLES_PER_CORE = 4
CM = 64
CIN = 256
COUT = 384
H = W = 64
PIX = H * W          # 4096
PAD = 4
WP = W + 2 * PAD     # 72
FSZ = WP * WP        # 5184
GUARD = 4            # front/back guard elems per FF slot (OOB garbage ok)
SLP = FSZ + 2 * GUARD  # 5192 slot pitch
SHIFTS = (0, 1, 2, 4, WP, 2 * WP, 4 * WP)   # FF slot shifts
SHIFT_SLOT = {s: i for i, s in enumerate(SHIFTS)}
NFF = len(SHIFTS)    # 7
HALF = 2048          # pixels per half (32 rows)
CHUNK = 512          # conv/out matmul N
SLAB = 1024
NSLAB = PIX // SLAB
# tap-psum chunks per half: 7-row pieces of the 32 rows
TAPCH = ((0, 7), (7, 7), (14, 7), (21, 7), (28, 4))

# taps: (branch, dy, dx); dil = (1, 2, 4)[branch]
DILS = (1, 2, 4)


def _tap_pairs():
    """Pair taps so each pair's flat-offset delta is one of SHIFTS[1:].
    Returns list of (t1, t2_or_None, shift) with t=(br, dy, dx);
    t2's flat offset == t1's + shift (shift==0 for singles)."""
    pairs = []
    # b0 (5x5, dil 1)
    for dy in range(-2, 3):
        pairs.append(((0, dy, -2), (0, dy, -1), 1))
        pairs.append(((0, dy, 0), (0, dy, 1), 1))
    pairs.append(((0, -2, 2), (0, -1, 2), WP))
    pairs.append(((0, 0, 2), (0, 1, 2), WP))
    pairs.append(((0, 2, 2), None, 0))
    # b1 (3x3, dil 2)
    for dy in range(-1, 2):
        pairs.append(((1, dy, -1), (1, dy, 0), 2))
    pairs.append(((1, -1, 1), (1, 0, 1), 2 * WP))
    pairs.append(((1, 1, 1), None, 0))
    # b2 (3x3, dil 4)
    for dy in range(-1, 2):
        pairs.append(((2, dy, -1), (2, dy, 0), 4))
    pairs.append(((2, -1, 1), (2, 0, 1), 4 * WP))
    pairs.append(((2, 1, 1), None, 0))
    return pairs


PAIRS = _tap_pairs()
NSLOT = 2 * len(PAIRS)          # diag bank slots (46)
BR_PAIRS = {br: [(j, p) for j, p in enumerate(PAIRS) if p[0][0] == br]
            for br in range(3)}
assert [len(BR_PAIRS[b]) for b in range(3)] == [13, 5, 5]


def _tap_off(t, row0):
    """Flat offset (within a slot, before the +GUARD base) of tap t's rhs
    for an output chunk starting at block row `row0`, extended col 0."""
    br, dy, dx = t
    dil = DILS[br]
    return (PAD + row0 + dy * dil) * WP + dx * dil


_PROGRAM_CACHE = {}


def _build_program():
    nc = bacc.Bacc("TRN2", target_bir_lowering=False, debug=False,
                   num_devices=N_CORES)

    x4 = nc.dram_tensor("x4", [SAMPLES_PER_CORE, CIN, PIX], BF16,
                        kind="ExternalInput").ap()
    wconv = nc.dram_tensor("wconv", [128, 512], BF16,
                           kind="ExternalInput").ap()
    wout12_d = nc.dram_tensor("wout12", [128, 3 * 128], BF16,
                              kind="ExternalInput").ap()
    wout1_d = nc.dram_tensor("wout1", [128, COUT], BF16,
                             kind="ExternalInput").ap()
    aT_d = nc.dram_tensor("aT", [128, NSLOT], F32, kind="ExternalInput").ap()
    bT_d = nc.dram_tensor("bT", [128, NSLOT], F32, kind="ExternalInput").ap()
    ident_d = nc.dram_tensor("ident", [128, 128], FP8,
                             kind="ExternalInput").ap()
    convb_d = nc.dram_tensor("convb", [128, 1], F32, kind="ExternalInput").ap()
    biasout_d = nc.dram_tensor("biasout", [128, 3], F32,
                               kind="ExternalInput").ap()
    # per-branch inverse tap scales (folded into branch evac)
    sinv_d = nc.dram_tensor("sinv", [128, 3], F32, kind="ExternalInput").ap()
    y4 = nc.dram_tensor("y4", [SAMPLES_PER_CORE, COUT, PIX], BF16,
                        kind="ExternalOutput").ap()

    with tile.TileContext(nc) as tc:
        with ExitStack() as ctx:
            consts = ctx.enter_context(tc.tile_pool(name="consts", bufs=1))
            xpool = ctx.enter_context(tc.tile_pool(name="xp", bufs=3))
            ffpool = ctx.enter_context(tc.tile_pool(name="ffp", bufs=1))
            dgpool = ctx.enter_context(tc.tile_pool(name="dgp", bufs=1))
            opool = ctx.enter_context(tc.tile_pool(name="op", bufs=2))
            outpool = ctx.enter_context(tc.tile_pool(name="outp", bufs=2))
            smalls = ctx.enter_context(tc.tile_pool(name="sm", bufs=2))
            ps_a = ctx.enter_context(
                tc.tile_pool(name="psa", bufs=1, space="PSUM"))
            ps_out = ctx.enter_context(
                tc.tile_pool(name="pso", bufs=1, space="PSUM"))

            # ---- constants (conv-critical ones first, on the scalar
            # queue; ACT table warmed immediately so the first conv evac
            # does not eat the 1.3us table load) ----
            warm = consts.tile([128, 1], F32, tag="warm")
            nc.gpsimd.memset(warm[:], 0.0)
            nc.scalar.activation(warm[:], warm[:], AF.Identity)
            wconv_t = consts.tile([128, 512], BF16, tag="wconv")
            nc.scalar.dma_start(wconv_t[:], wconv[:])
            convb = consts.tile([128, 1], F32, tag="convb")
            nc.scalar.dma_start(convb[:], convb_d[:])
            wout12_t = consts.tile([128, 3 * 128], BF16, tag="wout12")
            nc.gpsimd.dma_start(wout12_t[:], wout12_d[:])
            wout1_t = consts.tile([128, COUT], BF16, tag="wout1")
            nc.gpsimd.dma_start(wout1_t[:], wout1_d[:])
            aT = consts.tile([128, NSLOT], F32, tag="aT")
            nc.gpsimd.dma_start(aT[:], aT_d[:])
            bT = consts.tile([128, NSLOT], F32, tag="bT")
            nc.gpsimd.dma_start(bT[:], bT_d[:])
            ident = consts.tile([128, 128], FP8, tag="ident")
            nc.gpsimd.dma_start(ident[:], ident_d[:])
            biasout = consts.tile([128, 3], F32, tag="biasout")
            nc.gpsimd.dma_start(biasout[:], biasout_d[:])
            sinv = consts.tile([128, 3], F32, tag="sinv")
            nc.gpsimd.dma_start(sinv[:], sinv_d[:])

            # persistent FF tiles (one per block parity): 7 slots of padded
            # fp8 f (slot 0) and its shifted copies; pad borders zeroed once.
            ffs, banks = [], []
            zeros = consts.tile([128, PAD * WP], F32, tag="zeros")
            nc.gpsimd.memset(zeros[:], 0.0)
            for par in range(2):
                ff = ffpool.tile([128, NFF * SLP], FP8, tag=f"ff{par}")
                s0 = ff[:, GUARD:GUARD + FSZ]
                v = s0.rearrange("p (r c) -> p r c", c=WP)
                nc.vector.tensor_copy(s0[:, 0:PAD * WP], zeros[:])
                nc.vector.tensor_copy(s0[:, (PAD + H) * WP:FSZ], zeros[:])
                zv = zeros[:].rearrange("p (r c) -> p r c", c=PAD)
                nc.vector.tensor_copy(v[:, PAD:PAD + H, 0:PAD], zv[:, 0:H, :])
                nc.vector.tensor_copy(v[:, PAD:PAD + H, PAD + W:WP],
                                      zv[:, 0:H, :])
                # zero guards + shifted-slot tails once: stale SBUF there can
                # be Inf/NaN bit patterns, and 0 * Inf = NaN even through a
                # zero diag k-tile
                nc.gpsimd.memset(ff[:, 0:GUARD], 0.0)
                nc.gpsimd.memset(ff[:, GUARD + FSZ:SLP], 0.0)
                for si in range(1, NFF):
                    nc.gpsimd.memset(
                        ff[:, si * SLP + SLP - SHIFTS[si]:(si + 1) * SLP],
                        0.0)
                ffs.append(ff)
                bank = dgpool.tile([128, NSLOT, 128], FP8, tag=f"bank{par}")
                for j, (t1, t2, _sh) in enumerate(PAIRS):
                    if t2 is None:      # zero partner slot for single taps
                        nc.gpsimd.memset(bank[:, 2 * j + 1, :], 0.0)
                banks.append(bank)

            def ff_slot0_rows(ff, r0, nrows):
                """[p, nrows, 64] valid-interior view of slot0 (conv dst)."""
                v = ff[:, GUARD:GUARD + FSZ].rearrange("p (r c) -> p r c",
                                                       c=WP)
                return v[:, PAD + r0:PAD + r0 + nrows, PAD:PAD + W]

            def pair_rhs(ff, t1, shift, row0, ncols):
                """[p, 2, ncols] rhs AP: k-tile1 = slot0 @ off(t1),
                k-tile2 = slot(shift) @ same off (== f @ off+shift)."""
                off = GUARD + _tap_off(t1, row0)
                v = ff[:, off:off + ncols]
                u = v.unsqueeze(1).broadcast_to([128, 2, ncols])
                # singles (shift 0) pair with the zero diag; point k-tile2
                # at slot 1 — small or zero dim-1 strides fault the PE
                slot = SHIFT_SLOT[shift] or 1
                u.ap[1] = [slot * SLP, 2]
                return u

            # round-robin engine pickers for evac/diag work
            def rr(seq):
                i = [0]

                def pick():
                    e = seq[i[0] % len(seq)]
                    i[0] += 1
                    return e
                return pick

            # Pool/GPSIMD cannot read PSUM: psum evacs go to ACT/DVE only;
            # Pool absorbs SBUF-side work (diags, relu) instead.
            diag_eng = rr(["g", "v", "g", "g", "v"])
            evac_eng = rr(["a", "v"])
            out_eng = rr(["a", "v"])

            def emit_conv_loads(blk):
                """Issue all x-slab DMAs for a block (SP queue) so they
                prefetch ahead of the compute that consumes them."""
                n0 = 2 * blk
                slabs = []
                for q in range(NSLAB):
                    xts = {}
                    for kc in range(4):
                        xt = xpool.tile([128, SLAB], BF16,
                                        tag=f"x{blk % 2}_{kc}")
                        nc.sync.dma_start(
                            xt[:],
                            x4[n0:n0 + 2, kc * 64:(kc + 1) * 64,
                               q * SLAB:(q + 1) * SLAB])
                        xts[kc] = xt
                    slabs.append(xts)
                return slabs

            def emit_conv(blk, slabs):
                n0 = 2 * blk
                ff = ffs[blk % 2]
                bank = banks[blk % 2]

                gsums = smalls.tile([128, 8], F32, tag=f"gsums{blk}")
                for q in range(NSLAB):
                    xts = slabs[q]
                    for c in range(SLAB // CHUNK):
                        j = q * (SLAB // CHUNK) + c
                        ps = ps_a.tile([128, CHUNK], F32, tag=f"tap{j % 2}")
                        for kc in range(4):
                            nc.tensor.matmul(
                                ps[:],
                                wconv_t[:, kc * 128:(kc + 1) * 128],
                                xts[kc][:, c * CHUNK:(c + 1) * CHUNK],
                                start=(kc == 0), stop=(kc == 3))
                        dst = ff_slot0_rows(ff, 8 * j, 8)
                        nc.scalar.activation(
                            dst, ps[:], AF.Identity,
                            bias=convb[:, 0:1],
                            accum_out=gsums[:, j:j + 1])
                        # relu this 8-row strip right away (pads stay 0), so
                        # only the last strip sits on the conv->copies chain
                        st0_ = GUARD + (PAD + 8 * j) * WP
                        strip = ff[:, st0_:st0_ + 8 * WP]
                        if j % 2 == 0:
                            nc.vector.tensor_scalar_max(strip, strip, 0.0)
                        else:
                            nc.gpsimd.tensor_scalar_max(strip, strip, 0.0)

                # shifted copies via SBUF->SBUF DMA; issued from the Pool
                # SWDGE queue so the SP sequencer keeps prefetching x slabs
                cp_eng = [nc.sync, nc.scalar, nc.gpsimd]
                for si in range(1, NFF):
                    sh = SHIFTS[si]
                    cp_eng[(si - 1) % 3].dma_start(
                        ff[:, si * SLP:si * SLP + SLP - sh],
                        ff[:, sh:SLP])

                # g -> ktile -> diag bank
                gpre = smalls.tile([128, 1], F32, tag=f"gpre{blk}")
                nc.vector.tensor_reduce(gpre[:], gsums[:], op=ALU.add,
                                        axis=mybir.AxisListType.X)
                gt = smalls.tile([128, 1], F32, tag=f"g{blk}")
                nc.scalar.activation(gt[:], gpre[:], AF.Relu,
                                     scale=1.0 / PIX)
                ktile = smalls.tile([128, NSLOT], F32, tag=f"ktile{blk}")
                nc.vector.scalar_tensor_tensor(ktile[:], aT[:], gt[:, 0:1],
                                               bT[:], op0=ALU.mult,
                                               op1=ALU.add)
                nc.vector.tensor_scalar_min(ktile[:], ktile[:], 240.0)
                nc.vector.tensor_scalar_max(ktile[:], ktile[:], -240.0)

                for j, (t1, t2, _sh) in enumerate(PAIRS):
                    for half_, t in ((0, t1), (1, t2)):
                        if t is None:
                            continue
                        sl = 2 * j + half_
                        e = diag_eng()
                        if e == "v":
                            nc.vector.tensor_scalar_mul(
                                bank[:, sl, :], ident[:],
                                ktile[:, sl:sl + 1])
                        elif e == "g":
                            nc.gpsimd.tensor_scalar_mul(
                                bank[:, sl, :], ident[:],
                                ktile[:, sl:sl + 1])
                        else:
                            nc.scalar.activation(
                                bank[:, sl, :], ident[:], AF.Copy,
                                scale=ktile[:, sl:sl + 1])
                return (ff, bank, n0)

            def emit_stage(st, h):
                """One (blk, h): tap chunks and out chunks interleaved so PE
                always has matmul work while ACT/DVE drain psums.
                Interleave: t0 t1 o0 t2 o1 t3 o2 t4 o3 (out chunk oc needs
                branch cols through 512(oc+1), covered by tap chunks
                through ceil(512(oc+1)/448)-1)."""
                ff, bank, n0 = st
                o1t = opool.tile([128, HALF], BF16, tag="o1")
                Xt = opool.tile([128, HALF], BF16, tag="X")
                Yt = opool.tile([128, HALF], BF16, tag="Y")
                pi = [0]
                osbs = {}

                def tap_chunk(ci):
                    lr0, nr = TAPCH[ci]
                    row0 = 32 * h + lr0
                    ncols = nr * WP
                    for br in range(3):
                        ps = ps_a.tile([128, 512], F32,
                                       tag=f"tap{pi[0] % 2}")
                        pi[0] += 1
                        plist = BR_PAIRS[br]
                        for i, (j, (t1, t2, sh)) in enumerate(plist):
                            rhs = pair_rhs(ff, t1, sh, row0, ncols)
                            nc.tensor.matmul(
                                ps[:, 0:ncols], bank[:, 2 * j:2 * j + 2, :],
                                rhs, start=(i == 0),
                                stop=(i == len(plist) - 1),
                                perf_mode=DRMODE)
                        src = ps[:, 0:ncols].rearrange(
                            "p (r c) -> p r c", c=WP)[:, :, PAD:PAD + W]
                        c0 = lr0 * W
                        csl = slice(c0, c0 + nr * W)
                        if br == 0:
                            dsts = [(slice(0, 128), o1t[:, csl])]
                        elif br == 1:
                            dsts = [(slice(0, 64), Xt[0:64, csl]),
                                    (slice(64, 128), Yt[0:64, csl])]
                        else:
                            dsts = [(slice(0, 64), Xt[64:128, csl]),
                                    (slice(64, 128), Yt[64:128, csl])]
                        for psl, dst in dsts:
                            e = evac_eng()
                            sc = sinv[psl, br:br + 1]
                            if e == "a":
                                nc.scalar.activation(dst, src[psl], AF.Copy,
                                                     scale=sc)
                            else:
                                nc.vector.tensor_scalar_mul(dst, src[psl],
                                                            sc)

                def out_chunk(oc):
                    if oc % 2 == 0:
                        for mt in range(3):
                            for s in range(2):
                                osb_tile = outpool.tile(
                                    [128, 2 * CHUNK], BF16,
                                    tag=f"osb{mt}_{s}")
                                osbs[(mt, s)] = osb_tile
                    csl = slice(oc * CHUNK, (oc + 1) * CHUNK)
                    pss = {}
                    for mt in range(3):
                        for s, bt in ((0, Xt), (1, Yt)):
                            ps = ps_out.tile([128, CHUNK], F32,
                                             tag=f"out{s}_{mt}")
                            pss[(s, mt)] = ps
                            nc.tensor.matmul(
                                ps[:],
                                wout12_t[:, mt * 128:(mt + 1) * 128],
                                bt[:, csl], start=True, stop=False)
                    for mt in range(3):
                        for s in range(2):
                            sl = slice(64 * s, 64 * s + 64)
                            nc.tensor.matmul(
                                pss[(s, mt)][:],
                                wout1_t[sl, mt * 128:(mt + 1) * 128],
                                o1t[sl, csl], start=False, stop=True)
                    for mt in range(3):
                        for s in range(2):
                            dst = osbs[(mt, s)][:, (oc % 2) * CHUNK:
                                                (oc % 2 + 1) * CHUNK]
                            if out_eng() == "a":
                                nc.scalar.activation(
                                    dst, pss[(s, mt)][:], AF.Identity,
                                    bias=biasout[:, mt:mt + 1])
                            else:
                                nc.vector.scalar_tensor_tensor(
                                    dst, pss[(s, mt)][:], 1.0,
                                    biasout[:, mt:mt + 1]
                                    .broadcast_to([128, CHUNK]),
                                    op0=ALU.mult, op1=ALU.add)
                    px0 = h * HALF + oc * CHUNK
                    half_sl = slice((oc % 2) * CHUNK, (oc % 2 + 1) * CHUNK)
                    for mt in range(3):
                        for s in range(2):
                            dst = y4[n0 + s, mt * 128:(mt + 1) * 128,
                                     px0:px0 + CHUNK]
                            nc.sync.dma_start(dst, osbs[(mt, s)][:, half_sl])

                for kind, i in (("t", 0), ("t", 1), ("o", 0), ("t", 2),
                                ("o", 1), ("t", 3), ("o", 2), ("t", 4),
                                ("o", 3)):
                    if kind == "t":
                        tap_chunk(i)
                    else:
                        out_chunk(i)

            sl0 = emit_conv_loads(0)
            st0 = emit_conv(0, sl0)
            sl1 = emit_conv_loads(1)
            emit_stage(st0, 0)
            st1 = emit_conv(1, sl1)
            emit_stage(st0, 1)
            emit_stage(st1, 0)
            emit_stage(st1, 1)
    nc.compile()
    return nc


def _get_program():
    if "nc" not in _PROGRAM_CACHE:
        _PROGRAM_CACHE["nc"] = _build_program()
    return _PROGRAM_CACHE["nc"]


def kernel(x, conv_w, conv_b, ck_w, ck_b, ck2_w, ck2_b, ckd4_w, ckd4_b,
           kern_w, kern_b, kern2_w, kern2_b, kernd4_w, kernd4_b,
           fuse_w, fuse_b, fc_w, fc_b):
    import ml_dtypes
    x = np.asarray(x, dtype=np.float32)
    conv_w = np.asarray(conv_w, dtype=np.float32)
    conv_b = np.asarray(conv_b, dtype=np.float32)
    fuse_w = np.asarray(fuse_w, dtype=np.float32)
    fuse_b = np.asarray(fuse_b, dtype=np.float32)
    fc_w = np.asarray(fc_w, dtype=np.float32)
    fc_b = np.asarray(fc_b, dtype=np.float32)

    NB = x.shape[0]
    assert NB == N_CORES * SAMPLES_PER_CORE

    # tap affine coefficients per branch: k_t = a_t * g + b_t
    def fold(sw, sb, kw, kb):
        a = (float(sw) * np.asarray(kw)).astype(np.float32)
        b = (float(sw) * np.asarray(kb) + float(sb)).astype(np.float32)
        return a, b

    a1, b1 = fold(ck_w, ck_b, kern_w, kern_b)        # [25], 5x5 row-major
    a2, b2 = fold(ck2_w, ck2_b, kern2_w, kern2_b)    # [9]
    a3, b3 = fold(ckd4_w, ckd4_b, kernd4_w, kernd4_b)

    def coef(t):
        br, dy, dx = t
        if br == 0:
            return a1[(dy + 2) * 5 + (dx + 2)], b1[(dy + 2) * 5 + (dx + 2)]
        a, b = (a2, b2) if br == 1 else (a3, b3)
        return a[(dy + 1) * 3 + (dx + 1)], b[(dy + 1) * 3 + (dx + 1)]

    # per-branch power-of-2 prescale: bound |k| with g <= GMAX, keep
    # S*|k| <= 200 so fp8e4m3 never saturates
    GMAX = 1.0
    scales = []
    for br in range(3):
        taps = [coef(t1) for (t1, t2, _s) in PAIRS if t1[0] == br]
        taps += [coef(t2) for (t1, t2, _s) in PAIRS
                 if t2 is not None and t2[0] == br]
        bound = max(abs(a) * GMAX + abs(b) for a, b in taps)
        scales.append(2.0 ** np.floor(np.log2(200.0 / max(bound, 1e-30))))
    sinv = np.zeros((128, 3), np.float32)
    for br in range(3):
        sinv[:, br] = 1.0 / scales[br]

    aT = np.zeros((128, NSLOT), np.float32)
    bT = np.zeros((128, NSLOT), np.float32)
    for j, (t1, t2, _sh) in enumerate(PAIRS):
        for half_, t in ((0, t1), (1, t2)):
            if t is None:
                continue
            a, b = coef(t)
            s = scales[t[0]]
            aT[:, 2 * j + half_] = a * s
            bT[:, 2 * j + half_] = b * s

    # folded output weights W_i = fc_w[:, 128i:128(i+1)] @ fuse_w  [384, 64]
    Wi = [fc_w[:, 128 * i:128 * (i + 1)] @ fuse_w for i in range(3)]
    wout12 = np.zeros((128, 3 * 128), dtype=np.float32)
    wout12[0:64, :] = Wi[1].T.reshape(64, COUT)
    wout12[64:128, :] = Wi[2].T.reshape(64, COUT)
    wout12 = wout12.astype(ml_dtypes.bfloat16)
    wout1 = np.zeros((128, COUT), dtype=np.float32)
    wout1[0:64, :] = Wi[0].T
    wout1[64:128, :] = Wi[0].T
    wout1 = wout1.astype(ml_dtypes.bfloat16)
    bias_out = (fc_w @ np.tile(fuse_b, 3) + fc_b).astype(np.float32)
    biasout = bias_out.reshape(3, 128).T.copy()

    wconv = np.zeros((128, 512), dtype=np.float32)
    for kc in range(4):
        wt = conv_w[:, 64 * kc:64 * (kc + 1)].T
        wconv[0:64, 128 * kc:128 * kc + 64] = wt
        wconv[64:128, 128 * kc + 64:128 * (kc + 1)] = wt
    wconv = wconv.astype(ml_dtypes.bfloat16)

    convb = np.concatenate([conv_b, conv_b]).reshape(128, 1).astype(np.float32)
    ident = np.eye(128, dtype=np.float32).astype(ml_dtypes.float8_e4m3)

    nc = _get_program()
    in_maps = []
    xbf = x.reshape(NB, CIN, PIX).astype(ml_dtypes.bfloat16)
    for core in range(N_CORES):
        xs = xbf[core * SAMPLES_PER_CORE:(core + 1) * SAMPLES_PER_CORE]
        in_maps.append({
            "x4": np.ascontiguousarray(xs),
            "wconv": wconv, "wout12": wout12, "wout1": wout1,
            "aT": aT, "bT": bT, "ident": ident, "convb": convb,
            "biasout": biasout, "sinv": sinv,
        })
    res = run_bass_kernel_spmd(nc, in_maps, list(range(N_CORES)))
    out = np.empty((NB, COUT, H, W), dtype=np.float32)
    for core in range(N_CORES):
        out[core * SAMPLES_PER_CORE:(core + 1) * SAMPLES_PER_CORE] = (
            res.results[core]["y4"].reshape(SAMPLES_PER_CORE, COUT, H, W)
            .astype(np.float32))
    return out


# revision 20
# speedup vs baseline: 1.1901x; 1.0336x over previous
"""Trainium2 Bass kernel for nn_DIDAModuleD4 (dynamic depthwise conv module).

Data-parallel over batch: 32 samples -> 8 cores x 4 samples.
Per core, samples are processed in 2 blocks of 2 samples; each block maps the
2x64=128 (sample, channel) pairs onto the 128 SBUF partitions.

Math (per sample, with host-side weight folding):
  f   = relu(conv_w @ x + conv_b)                       [64, 4096]
  g   = relu(mean_px(conv_w @ x + conv_b))              [64]
  k_t = a_t * g + b_t            (43 taps, a/b host-folded scalars)
  o_i = sum_t k_t * shift_t(f)   (depthwise; 5x5, 3x3 d2, 3x3 d4)
  out = sum_i W_i @ o_i + bias_out                      [384, 4096]

All 43 depthwise taps run on the PE as fp8e4 DoubleRow matmuls, two taps per
matmul: f is stored as an fp8 padded tile (72x72 flat domain) plus 6 shifted
copies (SBUF->SBUF DMA, shifts 1/2/4/72/144/288); a tap pair (t, t+delta)
reads k-tiles (slot0@off, slot_delta@off) so the rhs AP is [p, 2, N] with a
large monotonic dim-1 stride (small strides fault the PE).  lhsT k-tiles are
fp8 diag(k_t) matrices built per block from ktile; odd taps pair with an
all-zero diag slot.  DoubleRow costs 0.5 PE cycles/output-column for 2 taps
vs 1.0 for one bf16 tap (4x).  Tap matmuls produce 7-row x 72-col psum
chunks; the 8 pad columns per row are skipped at evacuation (pad wraparound
reads land in neighbor-row pad zeros since |dx*dil| <= PAD).

k values (~2.5e-3) sit in fp8's subnormal range, so ktile is prescaled by a
host-computed power of two per branch and the branch evacuation divides it
back out.  conv (f32r block-diag, 2-sample) and output 1x1s (bf16) are as in
the bf16 baseline; evacuations are spread across ACT/DVE/Pool.
"""

import sys

if "/opt/trn_rl_repo" not in sys.path:
    sys.path.insert(0, "/opt/trn_rl_repo")

import numpy as np
from contextlib import ExitStack

from concourse import bass, mybir, tile, bacc
from concourse.bass_utils import run_bass_kernel_spmd

F32 = mybir.dt.float32
F32R = mybir.dt.float32r
BF16 = mybir.dt.bfloat16
FP8 = mybir.dt.float8e4
AF = mybir.ActivationFunctionType
ALU = mybir.AluOpType
DRMODE = mybir.MatmulPerfMode.DoubleRow

N_CORES = 8
SAMPLES_PER_CORE = 4
CM = 64
CIN = 256
COUT = 384
H = W = 64
PIX = H * W          # 4096
PAD = 4
WP = W + 2 * PAD     # 72
FSZ = WP * WP        # 5184
GUARD = 4            # front/back guard elems per FF slot (OOB garbage ok)
SLP = FSZ + 2 * GUARD  # 5192 slot pitch
SHIFTS = (0, 1, 2, 4, WP, 2 * WP, 4 * WP)   # FF slot shifts
SHIFT_SLOT = {s: i for i, s in enumerate(SHIFTS)}
NFF = len(SHIFTS)    # 7
HALF = 2048          # pixels per half (32 rows)
CHUNK = 512          # conv/out matmul N
SLAB = 1024
NSLAB = PIX // SLAB
# tap-psum chunks per half: 7-row pieces of the 32 rows
TAPCH = ((0, 7), (7, 7), (14, 7), (21, 7), (28, 4))

# taps: (branch, dy, dx); dil = (1, 2, 4)[branch]
DILS = (1, 2, 4)


def _tap_pairs():
    """Pair taps so each pair's flat-offset delta is one of SHIFTS[1:].
    Returns list of (t1, t2_or_None, shift) with t=(br, dy, dx);
    t2's flat offset == t1's + shift (shift==0 for singles)."""
    pairs = []
    # b0 (5x5, dil 1)
    for dy in range(-2, 3):
        pairs.append(((0, dy, -2), (0, dy, -1), 1))
        pairs.append(((0, dy, 0), (0, dy, 1), 1))
    pairs.append(((0, -2, 2), (0, -1, 2), WP))
    pairs.append(((0, 0, 2), (0, 1, 2), WP))
    pairs.append(((0, 2, 2), None, 0))
    # b1 (3x3, dil 2)
    for dy in range(-1, 2):
        pairs.append(((1, dy, -1), (1, dy, 0), 2))
    pairs.append(((1, -1, 1), (1, 0, 1), 2 * WP))
    pairs.append(((1, 1, 1), None, 0))
    # b2 (3x3, dil 4)
    for dy in range(-1, 2):
        pairs.append(((2, dy, -1), (2, dy, 0), 4))
    pairs.append(((2, -1, 1), (2, 0, 1), 4 * WP))
    pairs.append(((2, 1, 1), None, 0))
    return pairs


PAIRS = _tap_pairs()
NSLOT = 2 * len(PAIRS)          # diag bank slots (46)
BR_PAIRS = {br: [(j, p) for j, p in enumerate(PAIRS) if p[0][0] == br]
            for br in range(3)}
assert [len(BR_PAIRS[b]) for b in range(3)] == [13, 5, 5]


def _tap_off(t, row0):
    """Flat offset (within a slot, before the +GUARD base) of tap t's rhs
    for an output chunk starting at block row `row0`, extended col 0."""
    br, dy, dx = t
    dil = DILS[br]
    return (PAD + row0 + dy * dil) * WP + dx * dil


_PROGRAM_CACHE = {}


def _build_program():
    nc = bacc.Bacc("TRN2", target_bir_lowering=False, debug=False,
                   num_devices=N_CORES)

    x4 = nc.dram_tensor("x4", [SAMPLES_PER_CORE, CIN, PIX], BF16,
                        kind="ExternalInput").ap()
    wconv = nc.dram_tensor("wconv", [128, 512], BF16,
                           kind="ExternalInput").ap()
    wout12_d = nc.dram_tensor("wout12", [128, 3 * 128], BF16,
                              kind="ExternalInput").ap()
    wout1_d = nc.dram_tensor("wout1", [128, COUT], BF16,
                             kind="ExternalInput").ap()
    aT_d = nc.dram_tensor("aT", [128, NSLOT], F32, kind="ExternalInput").ap()
    bT_d = nc.dram_tensor("bT", [128, NSLOT], F32, kind="ExternalInput").ap()
    ident_d = nc.dram_tensor("ident", [128, 128], FP8,
                             kind="ExternalInput").ap()
    convb_d = nc.dram_tensor("convb", [128, 1], F32, kind="ExternalInput").ap()
    biasout_d = nc.dram_tensor("biasout", [128, 3], F32,
                               kind="ExternalInput").ap()
    # per-branch inverse tap scales (folded into branch evac)
    sinv_d = nc.dram_tensor("sinv", [128, 3], F32, kind="ExternalInput").ap()
    y4 = nc.dram_tensor("y4", [SAMPLES_PER_CORE, COUT, PIX], BF16,
                        kind="ExternalOutput").ap()

    with tile.TileContext(nc) as tc:
        with ExitStack() as ctx:
            consts = ctx.enter_context(tc.tile_pool(name="consts", bufs=1))
            xpool = ctx.enter_context(tc.tile_pool(name="xp", bufs=3))
            ffpool = ctx.enter_context(tc.tile_pool(name="ffp", bufs=1))
            dgpool = ctx.enter_context(tc.tile_pool(name="dgp", bufs=1))
            opool = ctx.enter_context(tc.tile_pool(name="op", bufs=2))
            outpool = ctx.enter_context(tc.tile_pool(name="outp", bufs=2))
            smalls = ctx.enter_context(tc.tile_pool(name="sm", bufs=2))
            ps_a = ctx.enter_context(
                tc.tile_pool(name="psa", bufs=1, space="PSUM"))
            ps_out = ctx.enter_context(
                tc.tile_pool(name="pso", bufs=1, space="PSUM"))

            # ---- constants (conv-critical ones first, on the scalar
            # queue; ACT table warmed immediately so the first conv evac
            # does not eat the 1.3us table load) ----
            warm = consts.tile([128, 1], F32, tag="warm")
            nc.gpsimd.memset(warm[:], 0.0)
            nc.scalar.activation(warm[:], warm[:], AF.Identity)
            wconv_t = consts.tile([128, 512], BF16, tag="wconv")
            nc.scalar.dma_start(wconv_t[:], wconv[:])
            convb = consts.tile([128, 1], F32, tag="convb")
            nc.scalar.dma_start(convb[:], convb_d[:])
            wout12_t = consts.tile([128, 3 * 128], BF16, tag="wout12")
            nc.gpsimd.dma_start(wout12_t[:], wout12_d[:])
            wout1_t = consts.tile([128, COUT], BF16, tag="wout1")
            nc.gpsimd.dma_start(wout1_t[:], wout1_d[:])
            aT = consts.tile([128, NSLOT], F32, tag="aT")
            nc.gpsimd.dma_start(aT[:], aT_d[:])
            bT = consts.tile([128, NSLOT], F32, tag="bT")
            nc.gpsimd.dma_start(bT[:], bT_d[:])
            ident = consts.tile([128, 128], FP8, tag="ident")
            nc.gpsimd.dma_start(ident[:], ident_d[:])
            biasout = consts.tile([128, 3], F32, tag="biasout")
            nc.gpsimd.dma_start(biasout[:], biasout_d[:])
            sinv = consts.tile([128, 3], F32, tag="sinv")
            nc.gpsimd.dma_start(sinv[:], sinv_d[:])

            # persistent FF tiles (one per block parity): 7 slots of padded
            # fp8 f (slot 0) and its shifted copies; pad borders zeroed once.
            ffs, banks = [], []
            zeros = consts.tile([128, PAD * WP], F32, tag="zeros")
            nc.gpsimd.memset(zeros[:], 0.0)
            for par in range(2):
                ff = ffpool.tile([128, NFF * SLP], FP8, tag=f"ff{par}")
                s0 = ff[:, GUARD:GUARD + FSZ]
                v = s0.rearrange("p (r c) -> p r c", c=WP)
                nc.vector.tensor_copy(s0[:, 0:PAD * WP], zeros[:])
                nc.vector.tensor_copy(s0[:, (PAD + H) * WP:FSZ], zeros[:])
                zv = zeros[:].rearrange("p (r c) -> p r c", c=PAD)
                nc.vector.tensor_copy(v[:, PAD:PAD + H, 0:PAD], zv[:, 0:H, :])
                nc.vector.tensor_copy(v[:, PAD:PAD + H, PAD + W:WP],
                                      zv[:, 0:H, :])
                # zero guards + shifted-slot tails once: stale SBUF there can
                # be Inf/NaN bit patterns, and 0 * Inf = NaN even through a
                # zero diag k-tile
                nc.gpsimd.memset(ff[:, 0:GUARD], 0.0)
                nc.gpsimd.memset(ff[:, GUARD + FSZ:SLP], 0.0)
                for si in range(1, NFF):
                    nc.gpsimd.memset(
                        ff[:, si * SLP + SLP - SHIFTS[si]:(si + 1) * SLP],
                        0.0)
                ffs.append(ff)
                bank = dgpool.tile([128, NSLOT, 128], FP8, tag=f"bank{par}")
                for j, (t1, t2, _sh) in enumerate(PAIRS):
                    if t2 is None:      # zero partner slot for single taps
                        nc.gpsimd.memset(bank[:, 2 * j + 1, :], 0.0)
                banks.append(bank)

            def ff_slot0_rows(ff, r0, nrows):
                """[p, nrows, 64] valid-interior view of slot0 (conv dst)."""
                v = ff[:, GUARD:GUARD + FSZ].rearrange("p (r c) -> p r c",
                                                       c=WP)
                return v[:, PAD + r0:PAD + r0 + nrows, PAD:PAD + W]

            def pair_rhs(ff, t1, shift, row0, ncols):
                """[p, 2, ncols] rhs AP: k-tile1 = slot0 @ off(t1),
                k-tile2 = slot(shift) @ same off (== f @ off+shift)."""
                off = GUARD + _tap_off(t1, row0)
                v = ff[:, off:off + ncols]
                u = v.unsqueeze(1).broadcast_to([128, 2, ncols])
                # singles (shift 0) pair with the zero diag; point k-tile2
                # at slot 1 — small or zero dim-1 strides fault the PE
                slot = SHIFT_SLOT[shift] or 1
                u.ap[1] = [slot * SLP, 2]
                return u

            # round-robin engine pickers for evac/diag work
            def rr(seq):
                i = [0]

                def pick():
                    e = seq[i[0] % len(seq)]
                    i[0] += 1
                    return e
                return pick

            # Pool/GPSIMD cannot read PSUM: psum evacs go to ACT/DVE only;
            # Pool absorbs SBUF-side work (diags, relu) instead.
            diag_eng = rr(["v", "g", "a", "v"])
            evac_eng = rr(["a", "v"])
            out_eng = rr(["a", "v"])

            def emit_conv_loads(blk):
                """Issue all x-slab DMAs for a block (SP queue) so they
                prefetch ahead of the compute that consumes them."""
                n0 = 2 * blk
                slabs = []
                for q in range(NSLAB):
                    xts = {}
                    for kc in range(4):
                        xt = xpool.tile([128, SLAB], BF16,
                                        tag=f"x{blk % 2}_{kc}")
                        nc.sync.dma_start(
                            xt[:],
                            x4[n0:n0 + 2, kc * 64:(kc + 1) * 64,
                               q * SLAB:(q + 1) * SLAB])
                        xts[kc] = xt
                    slabs.append(xts)
                return slabs

            def emit_conv(blk, slabs):
                n0 = 2 * blk
                ff = ffs[blk % 2]
                bank = banks[blk % 2]

                gsums = smalls.tile([128, 8], F32, tag=f"gsums{blk}")
                for q in range(NSLAB):
                    xts = slabs[q]
                    for c in range(SLAB // CHUNK):
                        j = q * (SLAB // CHUNK) + c
                        ps = ps_a.tile([128, CHUNK], F32, tag=f"tap{j % 2}")
                        for kc in range(4):
                            nc.tensor.matmul(
                                ps[:],
                                wconv_t[:, kc * 128:(kc + 1) * 128],
                                xts[kc][:, c * CHUNK:(c + 1) * CHUNK],
                                start=(kc == 0), stop=(kc == 3))
                        dst = ff_slot0_rows(ff, 8 * j, 8)
                        nc.scalar.activation(
                            dst, ps[:], AF.Identity,
                            bias=convb[:, 0:1],
                            accum_out=gsums[:, j:j + 1])
                        # relu this 8-row strip right away (pads stay 0), so
                        # only the last strip sits on the conv->copies chain
                        st0_ = GUARD + (PAD + 8 * j) * WP
                        strip = ff[:, st0_:st0_ + 8 * WP]
                        if j % 2 == 0:
                            nc.vector.tensor_scalar_max(strip, strip, 0.0)
                        else:
                            nc.gpsimd.tensor_scalar_max(strip, strip, 0.0)

                # shifted copies via SBUF->SBUF DMA; issued from the Pool
                # SWDGE queue so the SP sequencer keeps prefetching x slabs
                # each copy split in two: piece A covers everything the
                # h=0 tap chunks read (rows through 40 + margin), so the
                # first taps start before the full-tile copies finish
                CUT = GUARD + 40 * WP + 8
                cp_eng = [nc.sync, nc.scalar, nc.gpsimd]
                for si in range(1, NFF):
                    sh = SHIFTS[si]
                    q = cp_eng[(si - 1) % 3]
                    q.dma_start(ff[:, si * SLP:si * SLP + CUT],
                                ff[:, sh:sh + CUT])
                for si in range(1, NFF):
                    sh = SHIFTS[si]
                    q = cp_eng[si % 3]
                    q.dma_start(ff[:, si * SLP + CUT:si * SLP + SLP - sh],
                                ff[:, CUT + sh:SLP])

                # g -> ktile -> diag bank
                gpre = smalls.tile([128, 1], F32, tag=f"gpre{blk}")
                nc.vector.tensor_reduce(gpre[:], gsums[:], op=ALU.add,
                                        axis=mybir.AxisListType.X)
                gt = smalls.tile([128, 1], F32, tag=f"g{blk}")
                nc.scalar.activation(gt[:], gpre[:], AF.Relu,
                                     scale=1.0 / PIX)
                ktile = smalls.tile([128, NSLOT], F32, tag=f"ktile{blk}")
                nc.vector.scalar_tensor_tensor(ktile[:], aT[:], gt[:, 0:1],
                                               bT[:], op0=ALU.mult,
                                               op1=ALU.add)
                nc.vector.tensor_scalar_min(ktile[:], ktile[:], 240.0)
                nc.vector.tensor_scalar_max(ktile[:], ktile[:], -240.0)

                for j, (t1, t2, _sh) in enumerate(PAIRS):
                    for half_, t in ((0, t1), (1, t2)):
                        if t is None:
                            continue
                        sl = 2 * j + half_
                        e = diag_eng()
                        if e == "v":
                            nc.vector.tensor_scalar_mul(
                                bank[:, sl, :], ident[:],
                                ktile[:, sl:sl + 1])
                        elif e == "g":
                            nc.gpsimd.tensor_scalar_mul(
                                bank[:, sl, :], ident[:],
                                ktile[:, sl:sl + 1])
                        else:
                            nc.scalar.activation(
                                bank[:, sl, :], ident[:], AF.Copy,
                                scale=ktile[:, sl:sl + 1])
                return (ff, bank, n0)

            def emit_stage(st, h):
                """One (blk, h): tap chunks and out chunks interleaved so PE
                always has matmul work while ACT/DVE drain psums.
                Interleave: t0 t1 o0 t2 o1 t3 o2 t4 o3 (out chunk oc needs
                branch cols through 512(oc+1), covered by tap chunks
                through ceil(512(oc+1)/448)-1)."""
                ff, bank, n0 = st
                o1t = opool.tile([128, HALF], BF16, tag="o1")
                Xt = opool.tile([128, HALF], BF16, tag="X")
                Yt = opool.tile([128, HALF], BF16, tag="Y")
                pi = [0]
                osbs = {}

                def tap_chunk(ci):
                    lr0, nr = TAPCH[ci]
                    row0 = 32 * h + lr0
                    ncols = nr * WP
                    for br in range(3):
                        ps = ps_a.tile([128, 512], F32,
                                       tag=f"tap{pi[0] % 2}")
                        pi[0] += 1
                        plist = BR_PAIRS[br]
                        for i, (j, (t1, t2, sh)) in enumerate(plist):
                            rhs = pair_rhs(ff, t1, sh, row0, ncols)
                            nc.tensor.matmul(
                                ps[:, 0:ncols], bank[:, 2 * j:2 * j + 2, :],
                                rhs, start=(i == 0),
                                stop=(i == len(plist) - 1),
                                perf_mode=DRMODE)
                        src = ps[:, 0:ncols].rearrange(
                            "p (r c) -> p r c", c=WP)[:, :, PAD:PAD + W]
                        c0 = lr0 * W
                        csl = slice(c0, c0 + nr * W)
                        if br == 0:
                            dsts = [(slice(0, 128), o1t[:, csl])]
                        elif br == 1:
                            dsts = [(slice(0, 64), Xt[0:64, csl]),
                                    (slice(64, 128), Yt[0:64, csl])]
                        else:
                            dsts = [(slice(0, 64), Xt[64:128, csl]),
                                    (slice(64, 128), Yt[64:128, csl])]
                        for psl, dst in dsts:
                            e = evac_eng()
                            sc = sinv[psl, br:br + 1]
                            if e == "a":
                                nc.scalar.activation(dst, src[psl], AF.Copy,
                                                     scale=sc)
                            else:
                                nc.vector.tensor_scalar_mul(dst, src[psl],
                                                            sc)

                def out_chunk(oc):
                    if oc % 2 == 0:
                        for mt in range(3):
                            for s in range(2):
                                osb_tile = outpool.tile(
                                    [128, 2 * CHUNK], BF16,
                                    tag=f"osb{mt}_{s}")
                                osbs[(mt, s)] = osb_tile
                    csl = slice(oc * CHUNK, (oc + 1) * CHUNK)
                    pss = {}
                    for mt in range(3):
                        for s, bt in ((0, Xt), (1, Yt)):
                            ps = ps_out.tile([128, CHUNK], F32,
                                             tag=f"out{s}_{mt}")
                            pss[(s, mt)] = ps
                            nc.tensor.matmul(
                                ps[:],
                                wout12_t[:, mt * 128:(mt + 1) * 128],
                                bt[:, csl], start=True, stop=False)
                    for mt in range(3):
                        for s in range(2):
                            sl = slice(64 * s, 64 * s + 64)
                            nc.tensor.matmul(
                                pss[(s, mt)][:],
                                wout1_t[sl, mt * 128:(mt + 1) * 128],
                                o1t[sl, csl], start=False, stop=True)
                    for mt in range(3):
                        for s in range(2):
                            dst = osbs[(mt, s)][:, (oc % 2) * CHUNK:
                                                (oc % 2 + 1) * CHUNK]
                            if out_eng() == "a":
                                nc.scalar.activation(
                                    dst, pss[(s, mt)][:], AF.Identity,
                                    bias=biasout[:, mt:mt + 1])
                            else:
                                nc.vector.scalar_tensor_tensor(
                                    dst, pss[(s, mt)][:], 1.0,
                                    biasout[:, mt:mt + 1]
                                    .broadcast_to([128, CHUNK]),
                                    op0=ALU.mult, op1=ALU.add)
                    px0 = h * HALF + oc * CHUNK
                    half_sl = slice((oc % 2) * CHUNK, (oc % 2 + 1) * CHUNK)
                    for mt in range(3):
                        for s in range(2):
                            dst = y4[n0 + s, mt * 128:(mt + 1) * 128,
                                     px0:px0 + CHUNK]
                            nc.sync.dma_start(dst, osbs[(mt, s)][:, half_sl])

                for kind, i in (("t", 0), ("t", 1), ("o", 0), ("t", 2),
                                ("o", 1), ("t", 3), ("o", 2), ("t", 4),
                                ("o", 3)):
                    if kind == "t":
                        tap_chunk(i)
                    else:
                        out_chunk(i)

            sl0 = emit_conv_loads(0)
            st0 = emit_conv(0, sl0)
            sl1 = emit_conv_loads(1)
            emit_stage(st0, 0)
            st1 = emit_conv(1, sl1)
            emit_stage(st0, 1)
            emit_stage(st1, 0)
            emit_stage(st1, 1)
    nc.compile()
    return nc


def _get_program():
    if "nc" not in _PROGRAM_CACHE:
        _PROGRAM_CACHE["nc"] = _build_program()
    return _PROGRAM_CACHE["nc"]


def kernel(x, conv_w, conv_b, ck_w, ck_b, ck2_w, ck2_b, ckd4_w, ckd4_b,
           kern_w, kern_b, kern2_w, kern2_b, kernd4_w, kernd4_b,
           fuse_w, fuse_b, fc_w, fc_b):
    import ml_dtypes
    x = np.asarray(x, dtype=np.float32)
    conv_w = np.asarray(conv_w, dtype=np.float32)
    conv_b = np.asarray(conv_b, dtype=np.float32)
    fuse_w = np.asarray(fuse_w, dtype=np.float32)
    fuse_b = np.asarray(fuse_b, dtype=np.float32)
    fc_w = np.asarray(fc_w, dtype=np.float32)
    fc_b = np.asarray(fc_b, dtype=np.float32)

    NB = x.shape[0]
    assert NB == N_CORES * SAMPLES_PER_CORE

    # tap affine coefficients per branch: k_t = a_t * g + b_t
    def fold(sw, sb, kw, kb):
        a = (float(sw) * np.asarray(kw)).astype(np.float32)
        b = (float(sw) * np.asarray(kb) + float(sb)).astype(np.float32)
        return a, b

    a1, b1 = fold(ck_w, ck_b, kern_w, kern_b)        # [25], 5x5 row-major
    a2, b2 = fold(ck2_w, ck2_b, kern2_w, kern2_b)    # [9]
    a3, b3 = fold(ckd4_w, ckd4_b, kernd4_w, kernd4_b)

    def coef(t):
        br, dy, dx = t
        if br == 0:
            return a1[(dy + 2) * 5 + (dx + 2)], b1[(dy + 2) * 5 + (dx + 2)]
        a, b = (a2, b2) if br == 1 else (a3, b3)
        return a[(dy + 1) * 3 + (dx + 1)], b[(dy + 1) * 3 + (dx + 1)]

    # per-branch power-of-2 prescale: bound |k| with g <= GMAX, keep
    # S*|k| <= 200 so fp8e4m3 never saturates
    GMAX = 1.0
    scales = []
    for br in range(3):
        taps = [coef(t1) for (t1, t2, _s) in PAIRS if t1[0] == br]
        taps += [coef(t2) for (t1, t2, _s) in PAIRS
                 if t2 is not None and t2[0] == br]
        bound = max(abs(a) * GMAX + abs(b) for a, b in taps)
        scales.append(2.0 ** np.floor(np.log2(200.0 / max(bound, 1e-30))))
    sinv = np.zeros((128, 3), np.float32)
    for br in range(3):
        sinv[:, br] = 1.0 / scales[br]

    aT = np.zeros((128, NSLOT), np.float32)
    bT = np.zeros((128, NSLOT), np.float32)
    for j, (t1, t2, _sh) in enumerate(PAIRS):
        for half_, t in ((0, t1), (1, t2)):
            if t is None:
                continue
            a, b = coef(t)
            s = scales[t[0]]
            aT[:, 2 * j + half_] = a * s
            bT[:, 2 * j + half_] = b * s

    # folded output weights W_i = fc_w[:, 128i:128(i+1)] @ fuse_w  [384, 64]
    Wi = [fc_w[:, 128 * i:128 * (i + 1)] @ fuse_w for i in range(3)]
    wout12 = np.zeros((128, 3 * 128), dtype=np.float32)
    wout12[0:64, :] = Wi[1].T.reshape(64, COUT)
    wout12[64:128, :] = Wi[2].T.reshape(64, COUT)
    wout12 = wout12.astype(ml_dtypes.bfloat16)
    wout1 = np.zeros((128, COUT), dtype=np.float32)
    wout1[0:64, :] = Wi[0].T
    wout1[64:128, :] = Wi[0].T
    wout1 = wout1.astype(ml_dtypes.bfloat16)
    bias_out = (fc_w @ np.tile(fuse_b, 3) + fc_b).astype(np.float32)
    biasout = bias_out.reshape(3, 128).T.copy()

    wconv = np.zeros((128, 512), dtype=np.float32)
    for kc in range(4):
        wt = conv_w[:, 64 * kc:64 * (kc + 1)].T
        wconv[0:64, 128 * kc:128 * kc + 64] = wt
        wconv[64:128, 128 * kc + 64:128 * (kc + 1)] = wt
    wconv = wconv.astype(ml_dtypes.bfloat16)

    convb = np.concatenate([conv_b, conv_b]).reshape(128, 1).astype(np.float32)
    ident = np.eye(128, dtype=np.float32).astype(ml_dtypes.float8_e4m3)

    nc = _get_program()
    in_maps = []
    xbf = x.reshape(NB, CIN, PIX).astype(ml_dtypes.bfloat16)
    for core in range(N_CORES):
        xs = xbf[core * SAMPLES_PER_CORE:(core + 1) * SAMPLES_PER_CORE]
        in_maps.append({
            "x4": np.ascontiguousarray(xs),
            "wconv": wconv, "wout12": wout12, "wout1": wout1,
            "aT": aT, "bT": bT, "ident": ident, "convb": convb,
            "biasout": biasout, "sinv": sinv,
        })
    res = run_bass_kernel_spmd(nc, in_maps, list(range(N_CORES)))
    out = np.empty((NB, COUT, H, W), dtype=np.float32)
    for core in range(N_CORES):
        out[core * SAMPLES_PER_CORE:(core + 1) * SAMPLES_PER_CORE] = (
            res.results[core]["y4"].reshape(SAMPLES_PER_CORE, COUT, H, W)
            .astype(np.float32))
    return out
